# revision 14
# baseline (speedup 1.0000x reference)
"""Causal-attention (QKV projection + softmax(QK^T/sqrt(d))V) on 8 trn2 cores.

Contract: kernel(x, Wq, Wk, Wv) takes FULL inputs
  x [4, 4096, 768] f32, Wq/Wk/Wv [768, 128] f32
and returns the FULL output [4, 4096, 128] f32.

Sharding: 2 cores per batch. Core with parity h in {0,1} of batch b owns query
rows h::2 (perfect causal load balance). The host permutes the per-core input
to xT_p = concat(x[b, h::2], x[b, 1-h::2]).T so one compiled SPMD program runs
on every core; causality is enforced with per-core [128,128] triangular
additive-mask tiles applied only on the diagonal 128-key blocks.

v3 changes vs v2 (83.4us):
  - x shipped fp8e4 (half the input bytes) + a small f16 "head" (first 256
    rows of each parity) for exact early-row projections
  - QKV projections in fp8 DoubleRow (2 k-tiles per pass)
  - P = exp(s*SCALE - C) written fp8e4 (C=2.0 keeps max p ~103 < 240);
    AV matmul in fp8 DoubleRow (both key blocks of a pair in one matmul)
  - denominator via ones-weights matmuls on the PE accumulating into a
    [16,512] psum tile per q-tile (replaces the DVE sacc accumulation and
    the 1MB out_den DMA; host divide now reads a [1,2048] f32 den)
  - diagonal pairs use 4 dedicated pt tiles whose masked head regions are
    zeroed once and never rewritten, so AV/den run full-width DoubleRow
  - f16 carve-out for key blocks 0,1/16,17 vs queries [0,256): head-exact
    q/k/v + f16 pt + plain-mode AV/den protect all rows with <256 keys
"""
import numpy as np
import ml_dtypes

import concourse.bass as bass
import concourse.mybir as mybir
import concourse.tile as tile_mod
from concourse.tile import ScopedClock, VectorClock
from concourse.tile_sem_assignment import N_PROCS
from concourse.bass_utils import run_bass_kernel_spmd

f32 = mybir.dt.float32
f16 = mybir.dt.float16
f8 = mybir.dt.float8e4
E4 = ml_dtypes.float8_e4m3

B, S, D_IN, D = 4, 4096, 768, 128
N_DIN = D_IN // 128  # 6
TQ = 512             # queries per q-tile
NQ = S // 2          # queries per core
N_QT = NQ // TQ      # 4 q-tiles
SCALE = 1.0 / np.sqrt(np.float32(D))
CSHIFT = 2.0         # exp(s*SCALE - CSHIFT); num/den ratio is C-invariant
AF = mybir.ActivationFunctionType
DR = mybir.MatmulPerfMode.DoubleRow
N_WARM = 3

# ---------------------------------------------------------------------------
# Workarounds: the walrus build in this container accepts only ONE sync-wait
# command per instruction. TileContext's exit drain carries one wait per
# active proc, and Tile's sem assignment emits multi-wait instructions.
# Split both onto single-wait carrier instructions.
# ---------------------------------------------------------------------------


def _split_drain_and_barrier(self, tick_clock, wait_clock):
    gc = tick_clock.global_clock
    engs = [self.nc.sync, self.nc.scalar, self.nc.vector, self.nc.tensor]
    k = 0
    for p in range(N_PROCS):
        if gc[p] == 0:
            continue
        vc = VectorClock([gc[q] if q == p else 0 for q in range(N_PROCS)])
        d = engs[k % len(engs)].drain()
        k += 1
        wait_clock.add_sem_waits(d.ins, ScopedClock({None: vc}))
    self.nc.all_engine_barrier()
    assert self.sems is not None
    popped = self.nc._tile_sem_poison_stack.pop()
    assert popped is self._sem_poison
    self.nc.clear_and_free_semaphores(list(self.sems.allocated().values()))
    self.nc.all_engine_barrier()


tile_mod.TileContext._drain_and_barrier = _split_drain_and_barrier


def _split_waits(nc, max_waits=1):
    for fn in nc.m.functions:
        for bb in fn.blocks:
            insts = bb.instructions
            if not any(
                i.sync_info and i.sync_info.on_wait
                and len(i.sync_info.on_wait) > max_waits
                for i in insts
            ):
                continue
            new = []
            for inst in insts:
                si = inst.sync_info
                ow = list(si.on_wait) if si and si.on_wait else []
                if len(ow) > max_waits:
                    excess, keep = ow[:-max_waits], ow[-max_waits:]
                    for j, w in enumerate(excess):
                        new.append(
                            mybir.InstEventSemaphore(
                                name=f"{inst.name}-wsplit{j}",
                                engine=inst.engine,
                                ins=[],
                                outs=[],
                                sync_info=mybir.SyncInfo(
                                    on_wait=[w], on_update=[]
                                ),
                            )
                        )
                    inst.sync_info = mybir.SyncInfo(
                        on_wait=keep, on_update=list(si.on_update or [])
                    )
                new.append(inst)
            bb.instructions = new


# ---------------------------------------------------------------------------
# Device program
# ---------------------------------------------------------------------------


def _build():
    nc = bass.Bass()
    x8 = nc.declare_dram_parameter("x8", [D_IN, S], f8, isOutput=False)
    xh = nc.declare_dram_parameter("xh", [D_IN, 512], f16, isOutput=False)
    W8 = nc.declare_dram_parameter("W8", [128, 9 * 256], f8, isOutput=False)
    Wh = nc.declare_dram_parameter("Wh", [128, N_DIN * 3 * D], f16,
                                   isOutput=False)
    mask = nc.declare_dram_parameter("mask", [128, 4 * 128], f16,
                                     isOutput=False)
    out_num = nc.declare_dram_parameter("out_num", [D, NQ], f16, isOutput=True)
    out_den = nc.declare_dram_parameter("out_den", [1, NQ], f32, isOutput=True)

    with tile_mod.TileContext(nc) as tc:
        with (
            tc.tile_pool(name="persist", bufs=1) as persist,
            tc.tile_pool(name="work", bufs=6) as work,
            tc.tile_pool(name="outp", bufs=2) as outp,
            tc.tile_pool(name="ps_big", bufs=2, space="PSUM") as ps_big,
            tc.tile_pool(name="ps_out", bufs=1, space="PSUM") as ps_out,
            tc.tile_pool(name="ps_sml", bufs=2, space="PSUM") as ps_sml,
            tc.tile_pool(name="ps_den", bufs=1, space="PSUM") as ps_den,
        ):
            x_all8 = persist.tile([128, N_DIN, S], f8, tag="x_all8")
            xh_all = persist.tile([128, N_DIN, 512], f16, tag="xh_all")
            w8_all = persist.tile([128, 9, 2, 128], f8, tag="w8_all")
            wh_all = persist.tile([128, N_DIN * 3 * D], f16, tag="wh_all")
            m_all = persist.tile([128, 4 * 128], f16, tag="m_all")
            kt_sb = [persist.tile([128, 512], f16, tag=f"kt{c}", name=f"kt{c}")
                     for c in range(S // 512)]
            qt_sb = [persist.tile([128, TQ], f16, tag=f"qt{t}", name=f"qt{t}")
                     for t in range(N_QT)]
            # v8_sb[g][p, j, d] = v[key 128*(4g+j)+p, d] in fp8
            v8_sb = [persist.tile([128, 4, 128], f8, tag=f"v{g}", name=f"v{g}")
                     for g in range(S // 512)]
            # exact f16 v for key blocks 0,1 (own) and 16,17 (other)
            v16h = [persist.tile([128, 2, 128], f16, tag=f"vh{i}",
                                 name=f"vh{i}") for i in range(2)]
            # dedicated diag pt tiles; masked heads zeroed once, then only
            # the live region is ever rewritten -> full-width DR AV/den safe.
            # types: 0=A-own(los 0,128) 1=B-own(256,384) 2=A-oth 3=B-oth
            ptd = [persist.tile([128, 2, TQ], f8, tag=f"ptd{i}",
                                name=f"ptd{i}") for i in range(4)]
            # f16 carve-out pt for t=0 pairs kp=0 / kp=16
            pt16 = [persist.tile([128, 2, TQ], f16, tag=f"pt16_{i}",
                                 name=f"pt16_{i}") for i in range(2)]
            ones8 = persist.tile([128, 2, 128], f8, tag="ones8")
            ones16 = persist.tile([128, 128], f16, tag="ones16")
            biasC = persist.tile([128, 1], f32, tag="biasC")
            warm_sb = persist.tile([128, 1024], f16, tag="warm")

            # W8 host layout: [K|V|Q] blocks, 3 di-pairs each: idx = 3*b+m
            def w8_k(m):
                return w8_all[:, 0 + m]

            def w8_v(m):
                return w8_all[:, 3 + m]

            def w8_q(m):
                return w8_all[:, 6 + m]

            # Wh host layout: [K|V|Q], di-major inside
            def wh_k(di):
                return wh_all[:, 128 * di:128 * (di + 1)]

            def wh_v(di):
                return wh_all[:, 768 + 128 * di:768 + 128 * (di + 1)]

            def wh_q(di):
                return wh_all[:, 1536 + 128 * di:1536 + 128 * (di + 1)]

            tri = [m_all[:, 0:128], m_all[:, 128:256]]  # half1, half2
            ident = m_all[:, 256:384]
            fullm = m_all[:, 384:512]  # all -1000

            # input DMAs. sync: W8 + mask (+ per-tile outputs later);
            # vector: xh + Wh (head projections are the first real PE work);
            # gpsimd: x8 column waves (per-di for wave(0,0)).
            # sync ring (priority FIFO): the first computable work is the
            # fp8 t=0 own-half projection (W8 + wave(0,0)), so those bytes go
            # first; the f16 head inputs follow, then the later x8 waves.
            xsrc = x8.rearrange("(d p) c -> p d c", p=128)
            half = S // 2
            nc.sync.dma_start(out=w8_all[:], in_=W8.rearrange(
                "p (i a b) -> p i a b", i=9, a=2))
            nc.sync.dma_start(out=m_all[:], in_=mask[:])
            for di in range(N_DIN):  # wave(0,0) per-di
                nc.sync.dma_start(
                    out=x_all8[:, di, 0:512], in_=xsrc[:, di, 0:512]
                )
            nc.sync.dma_start(out=xh_all[:], in_=xh.rearrange(
                "(d p) c -> p d c", p=128))
            nc.sync.dma_start(out=wh_all[:], in_=Wh[:])

            def x_wave(t, h, eng):
                lo = 512 * t + half * h
                eng.dma_start(
                    out=x_all8[:, :, lo:lo + 512], in_=xsrc[:, :, lo:lo + 512]
                )

            nc.gpsimd.memset(warm_sb[:], 0.0)
            x_wave(0, 1, nc.sync)
            for t in (1, 2, 3):
                x_wave(t, 0, nc.sync)
                x_wave(t, 1, nc.sync)
            nc.gpsimd.memset(biasC[:], -float(CSHIFT))
            # B-diag pt heads [0:256) stay permanently zero (exp only ever
            # writes [256:512)); A-diag pairs write their full range
            for i in (1, 3):
                nc.gpsimd.memset(ptd[i][:, :, 0:256], 0.0)
            nc.gpsimd.memset(ones8[:], 1.0)
            nc.gpsimd.memset(ones16[:], 1.0)

            # PE pre-warm bridging the input-DMA wait (HAM + pstate
            # ramp). 1024-col matmuls into the (preamble-idle) score psum
            # pool keep duty high despite the 2-buf WAR rotation.
            for i in range(N_WARM):
                psw = ps_big.tile([128, 2, TQ], f32, tag="big",
                                  name=f"warm{i}")
                for s_ in (0, 1):
                    nc.tensor.matmul(
                        psw[:, s_, :], lhsT=warm_sb[:, 0:128],
                        rhs=warm_sb[:, 512 * s_:512 * (s_ + 1)],
                        start=True, stop=True,
                    )

            def x8_cols(m, c0, c1):
                return x_all8[:, 2 * m:2 * m + 2, c0:c1]

            # ---- head (f16-exact) projections: deps only on xh + Wh ----
            def project_heads():
                # kt0[:,0:256], kt4[:,0:256], qt0[:,0:256] and v16h from the
                # f16 head (own rows 0:256 = xh cols 0:256, other = 256:512)
                for dst, wsel, hcol in (
                    (kt_sb[0], wh_k, 0), (kt_sb[4], wh_k, 256),
                    (qt_sb[0], wh_q, 0),
                ):
                    ps = ps_sml.tile([128, 512], f32, tag="sml",
                                     name=f"ph{hcol}_{dst.name}")
                    for di in range(N_DIN):
                        nc.tensor.matmul(
                            ps[:, 0:256], lhsT=wsel(di),
                            rhs=xh_all[:, di, hcol:hcol + 256],
                            start=(di == 0), stop=(di == N_DIN - 1),
                        )
                    nc.vector.tensor_copy(dst[:, 0:256], ps[:, 0:256])
                for i in range(2):  # v16h own/other
                    ps = ps_sml.tile([128, 512], f32, tag="sml",
                                     name=f"phv{i}")
                    for j in range(2):
                        c0 = 256 * i + 128 * j
                        for di in range(N_DIN):
                            nc.tensor.matmul(
                                ps[:, 128 * j:128 * (j + 1)],
                                lhsT=xh_all[:, di, c0:c0 + 128],
                                rhs=wh_v(di),
                                start=(di == 0), stop=(di == N_DIN - 1),
                            )
                    nc.vector.tensor_copy(v16h[i][:], ps[:, 0:256])

            # ---- fp8 DoubleRow projections ----
            def project_kt8(c):
                lo = 256 if c in (0, 4) else 0  # head owns [0:256)
                ps = ps_sml.tile([128, 512], f32, tag="sml", name=f"pkt{c}")
                for m in range(3):
                    nc.tensor.matmul(
                        ps[:, lo:512], lhsT=w8_k(m),
                        rhs=x8_cols(m, 512 * c + lo, 512 * (c + 1)),
                        start=(m == 0), stop=(m == 2), perf_mode=DR,
                    )
                nc.vector.tensor_copy(kt_sb[c][:, lo:512], ps[:, lo:512])

            def project_qt8(t):
                lo = 256 if t == 0 else 0
                ps = ps_sml.tile([128, 512], f32, tag="sml", name=f"pqt{t}")
                for m in range(3):
                    nc.tensor.matmul(
                        ps[:, lo:512], lhsT=w8_q(m),
                        rhs=x8_cols(m, TQ * t + lo, TQ * (t + 1)),
                        start=(m == 0), stop=(m == 2), perf_mode=DR,
                    )
                nc.vector.tensor_copy(qt_sb[t][:, lo:512], ps[:, lo:512])

            def project_v8_group(g):
                ps = ps_sml.tile([128, 512], f32, tag="sml", name=f"pv{g}")
                for j in range(4):
                    k = 4 * g + j
                    for m in range(3):
                        nc.tensor.matmul(
                            ps[:, 128 * j:128 * (j + 1)],
                            lhsT=x8_cols(m, 128 * k, 128 * (k + 1)),
                            rhs=w8_v(m),
                            start=(m == 0), stop=(m == 2), perf_mode=DR,
                        )
                nc.vector.tensor_copy(v8_sb[g][:], ps[:])

            def proj_ops(t):
                """Projection op closures for q-tile t (emitted one tile
                early). For the last tile some projections are deferred into
                its own pair loop (late_ops) as PE filler under the final
                exp chain."""
                if t >= N_QT:
                    return []
                if t == N_QT - 1:
                    return [
                        lambda: project_qt8(t),
                        lambda: project_kt8(t),
                    ]
                return [
                    lambda: project_kt8(t),
                    lambda: project_kt8(N_QT + t),
                    lambda: project_v8_group(t),
                    lambda: project_v8_group(N_QT + t),
                    lambda: project_qt8(t),
                ]

            def fill_ops(t):
                if t == 0:
                    return [
                        lambda: project_kt8(N_QT),
                        lambda: project_v8_group(N_QT),
                        project_heads,
                        lambda: project_qt8(1),
                        lambda: project_kt8(1),
                        lambda: project_v8_group(1),
                    ]
                if t == 1:
                    return [
                        lambda: project_kt8(N_QT + 1),
                        lambda: project_v8_group(N_QT + 1),
                    ] + proj_ops(2)
                return proj_ops(t + 1)

            def late_ops(t):
                if t != N_QT - 1:
                    return []
                return [
                    (3, lambda: project_v8_group(t)),
                    (9, lambda: project_kt8(N_QT + t)),
                    (11, lambda: project_v8_group(N_QT + t)),
                ]

            # ---- preamble projection schedule: only the own-half fp8
            # projections (W8 + wave(0,0) deps); everything needing later
            # DMA waves or xh/Wh is interleaved into tile 0's pair loop ----
            project_kt8(0)
            project_qt8(0)
            project_v8_group(0)

            n_kt_half = NQ // 128  # 16

            for t in range(N_QT):
                pairs = [2 * j for j in range(2 * (t + 1))] + [
                    n_kt_half + 2 * j for j in range(2 * (t + 1))
                ]
                if t == 0:
                    # B-diag pairs first: they depend only on the fp8
                    # projections, giving the xh/Wh-dependent carve pairs
                    # (kp=0/16) time to have their head inputs land
                    pairs = [2, n_kt_half + 2, 0, n_kt_half]
                n = len(pairs)

                def pair_info(kp):
                    half2 = kp >= n_kt_half
                    rel = kp - n_kt_half if half2 else kp
                    diag = 4 * t <= rel < 4 * t + 4
                    los = (
                        [128 * (rel - 4 * t), 128 * (rel - 4 * t + 1)]
                        if diag else [0, 0]
                    )
                    carve = t == 0 and rel == 0
                    return half2, diag, los, carve

                def emit_scores(kp, name):
                    # non-carve diag pairs: both s_ cover [lo0:512) so exp is
                    # a single 3D slice; s_=1's extra 128 cols get a full
                    # -1000 mask (exp -> 0). carve keeps per-s_ ranges.
                    half2, diag, los, carve = pair_info(kp)
                    ps = ps_big.tile([128, 2, TQ], f32, tag="big", name=name)
                    for s_ in (0, 1):
                        kt = kp + s_
                        lo = los[s_] if (carve or not diag) else los[0]
                        nc.tensor.matmul(
                            ps[:, s_, lo:TQ],
                            lhsT=kt_sb[kt // 4][:, 128 * (kt % 4):128 * (kt % 4 + 1)],
                            rhs=qt_sb[t][:, lo:TQ],
                            start=True,
                            stop=not diag,
                            skip_group_check=diag,
                        )
                        if diag:
                            if s_ == 1 and not carve:
                                nc.tensor.matmul(
                                    ps[:, s_, lo:lo + 128],
                                    lhsT=ident,
                                    rhs=fullm,
                                    start=False,
                                    stop=False,
                                    skip_group_check=True,
                                )
                            nc.tensor.matmul(
                                ps[:, s_, los[s_]:los[s_] + 128],
                                lhsT=ident,
                                rhs=tri[1 if half2 else 0],
                                start=False,
                                stop=True,
                                skip_group_check=True,
                            )
                    return ps

                # software pipeline: scores one pair ahead of exp/AV
                fill = fill_ops(t)
                fill_done = 0
                late = list(late_ops(t))

                po = ps_out.tile([128, TQ], f32, tag="out", name=f"po{t}")
                dn = ps_den.tile([128, TQ], f32, tag="den", name=f"dn{t}")

                fa = [True]
                fd = [True]

                def emit_avden(kp, pt, last):
                    half2, diag, los, carve = pair_info(kp)
                    g, j = kp // 4, kp % 4
                    if carve:
                        vh = v16h[1 if half2 else 0]
                        for s_ in (0, 1):
                            lo = los[s_]
                            nc.tensor.matmul(
                                po[:, lo:TQ], lhsT=vh[:, s_, :],
                                rhs=pt[:, s_, lo:TQ],
                                start=fa[0], stop=False,
                                skip_group_check=True,
                            )
                            fa[0] = False
                            nc.tensor.matmul(
                                dn[:, lo:TQ], lhsT=ones16[:],
                                rhs=pt[:, s_, lo:TQ],
                                start=fd[0], stop=False,
                                skip_group_check=True,
                            )
                            fd[0] = False
                    else:
                        nc.tensor.matmul(
                            po[:], lhsT=v8_sb[g][:, j:j + 2, :], rhs=pt[:],
                            start=fa[0], stop=last,
                            perf_mode=DR, skip_group_check=True,
                        )
                        fa[0] = False
                        nc.tensor.matmul(
                            dn[:], lhsT=ones8[:], rhs=pt[:],
                            start=fd[0], stop=last,
                            perf_mode=DR, skip_group_check=True,
                        )
                        fd[0] = False

                def emit_exp(kp, ps, name):
                    half2, diag, los, carve = pair_info(kp)
                    if carve:
                        pt = pt16[1 if half2 else 0]
                        for s_ in (0, 1):
                            lo = los[s_]
                            nc.scalar.activation(
                                pt[:, s_, lo:TQ], ps[:, s_, lo:TQ],
                                AF.Exp, scale=float(SCALE), bias=biasC[:],
                            )
                    elif diag and los[0]:
                        # B-diag: dedicated tile, head [0:256) stays 0
                        pt = ptd[3 if half2 else 1]
                        lo0 = los[0]
                        nc.scalar.activation(
                            pt[:, :, lo0:TQ], ps[:, :, lo0:TQ],
                            AF.Exp, scale=float(SCALE), bias=biasC[:],
                        )
                    else:
                        # off-diag and A-diag: full-width single exp
                        pt = work.tile([128, 2, TQ], f8, tag="pt", name=name)
                        nc.scalar.activation(
                            pt[:], ps[:], AF.Exp, scale=float(SCALE),
                            bias=biasC[:],
                        )
                    return pt

                while late and late[0][0] <= 0:
                    late.pop(0)[1]()
                ps_q = [emit_scores(pairs[0], f"s{t}_0")]
                pend = []  # (kp, pt) with AV/den deferred by one step
                for i, kp in enumerate(pairs):
                    # AV/den run one pair behind their exp: hides the exp
                    # latency and the po/dn WAR at tile boundaries
                    if pend:
                        pkp, ppt = pend.pop(0)
                        emit_avden(pkp, ppt, False)
                    if t == 0:
                        # t=0 fills carry this tile's own late-DMA deps
                        # (other-half projections, heads) and must precede
                        # the score emits that read them
                        want = ((i + 1) * len(fill)) // n
                        while fill_done < want:
                            fill[fill_done]()
                            fill_done += 1
                    if i + 1 < n:
                        ps_q.append(emit_scores(pairs[i + 1], f"s{t}_{i + 1}"))
                    if t > 0:
                        want = ((i + 1) * len(fill)) // n
                        while fill_done < want:
                            fill[fill_done]()
                            fill_done += 1
                    while late and late[0][0] <= i + 1:
                        late.pop(0)[1]()
                    ps = ps_q.pop(0)
                    pt = emit_exp(kp, ps, f"p{t}_{kp}")
                    pend.append((kp, pt))
                pkp, ppt = pend.pop(0)
                emit_avden(pkp, ppt, True)
                ob = outp.tile([128, TQ], f16, tag="ob", name=f"ob{t}")
                db = outp.tile([1, TQ], f32, tag="db", name=f"db{t}")
                nc.vector.tensor_copy(db[:], dn[0:1, :])
                nc.vector.tensor_copy(ob[:], po[:])
                nc.sync.dma_start(
                    out=out_den[:, TQ * t:TQ * (t + 1)], in_=db[:]
                )
                nc.sync.dma_start(out=out_num[:, TQ * t:TQ * (t + 1)], in_=ob[:])
    _split_waits(nc)
    return nc


_NC_CACHE = []


def _get_nc():
    if not _NC_CACHE:
        _NC_CACHE.append(_build())
    return _NC_CACHE[0]


def _host_inputs(x, Wq, Wk, Wv):
    # W8 layout: [K|V|Q] blocks, each [128, 3 pairs, 2, 128] -> [128, 2304]
    def blk8(M):
        return (M.astype(np.float32).reshape(3, 2, 128, D)
                .transpose(2, 0, 1, 3).reshape(128, 768))

    W8 = np.ascontiguousarray(
        np.concatenate([blk8(Wk), blk8(Wv), blk8(Wq)], axis=1)
    ).astype(E4)

    # Wh layout: [K|V|Q], di-major inside (f16)
    def blkh(M):
        return M.astype(np.float16).reshape(N_DIN, 128, D).transpose(1, 0, 2)

    Wh = np.ascontiguousarray(
        np.concatenate([blkh(Wk), blkh(Wv), blkh(Wq)], axis=1)
        .reshape(128, N_DIN * 3 * D)
    )
    u = np.arange(128)[:, None]
    i = np.arange(128)[None, :]
    masks = {}
    for h in (0, 1):
        tri1 = (u <= i).astype(np.float32)          # own-parity half
        tri2 = (u <= i - 1 + h).astype(np.float32)  # other-parity half
        ma = np.concatenate(
            [(tri1 - 1.0) * 1000.0, (tri2 - 1.0) * 1000.0,
             np.eye(128, dtype=np.float32),
             np.full((128, 128), -1000.0, dtype=np.float32)], axis=1
        )
        masks[h] = np.ascontiguousarray(ma).astype(np.float16)
    in_maps = []
    for c in range(2 * B):
        b, h = divmod(c, 2)
        xp = np.concatenate([x[b, h::2], x[b, 1 - h::2]], axis=0)  # [S, 768]
        x8_p = np.ascontiguousarray(xp.T).astype(E4)  # [768, S]
        xh_p = np.ascontiguousarray(
            np.concatenate([xp[0:256], xp[2048:2304]], axis=0).T
        ).astype(np.float16)  # [768, 512]
        in_maps.append({"x8": x8_p, "xh": xh_p, "W8": W8, "Wh": Wh,
                        "mask": masks[h]})
    return in_maps


def kernel(x, Wq, Wk, Wv):
    x = np.asarray(x, np.float32)
    Wq = np.asarray(Wq, np.float32)
    Wk = np.asarray(Wk, np.float32)
    Wv = np.asarray(Wv, np.float32)
    nc = _get_nc()
    in_maps = _host_inputs(x, Wq, Wk, Wv)
    res = run_bass_kernel_spmd(nc, in_maps, list(range(2 * B)))
    out = np.empty((B, S, D), np.float32)
    for c in range(2 * B):
        b, h = divmod(c, 2)
        num = res.results[c]["out_num"].astype(np.float32)  # [128, NQ]
        den = res.results[c]["out_den"][0]       # [NQ] f32
        out[b, h::2, :] = (num / den[None, :]).T
    return out


# revision 15
# speedup vs baseline: 1.0843x; 1.0843x over previous
"""Causal-attention (QKV projection + softmax(QK^T/sqrt(d))V) on 8 trn2 cores.

Contract: kernel(x, Wq, Wk, Wv) takes FULL inputs
  x [4, 4096, 768] f32, Wq/Wk/Wv [768, 128] f32
and returns the FULL output [4, 4096, 128] f32.

Sharding: 2 cores per batch. Core with parity h in {0,1} of batch b owns query
rows h::2 (perfect causal load balance). The host permutes the per-core input
to xT_p = concat(x[b, h::2], x[b, 1-h::2]).T so one compiled SPMD program runs
on every core; causality is enforced with per-core [128,128] triangular
additive-mask tiles applied only on the diagonal 128-key blocks.

v3 changes vs v2 (83.4us):
  - x shipped fp8e4 (half the input bytes) + a small f16 "head" (first 256
    rows of each parity) for exact early-row projections
  - QKV projections in fp8 DoubleRow (2 k-tiles per pass)
  - P = exp(s*SCALE - C) written fp8e4 (C=2.0 keeps max p ~103 < 240);
    AV matmul in fp8 DoubleRow (both key blocks of a pair in one matmul)
  - denominator via ones-weights matmuls on the PE accumulating into a
    [16,512] psum tile per q-tile (replaces the DVE sacc accumulation and
    the 1MB out_den DMA; host divide now reads a [1,2048] f32 den)
  - diagonal pairs use 4 dedicated pt tiles whose masked head regions are
    zeroed once and never rewritten, so AV/den run full-width DoubleRow
  - f16 carve-out for key blocks 0,1/16,17 vs queries [0,256): head-exact
    q/k/v + f16 pt + plain-mode AV/den protect all rows with <256 keys
"""
import numpy as np
import ml_dtypes

import concourse.bass as bass
import concourse.mybir as mybir
import concourse.tile as tile_mod
from concourse.tile import ScopedClock, VectorClock
from concourse.tile_sem_assignment import N_PROCS
from concourse.bass_utils import run_bass_kernel_spmd

f32 = mybir.dt.float32
f16 = mybir.dt.float16
f8 = mybir.dt.float8e4
E4 = ml_dtypes.float8_e4m3

B, S, D_IN, D = 4, 4096, 768, 128
N_DIN = D_IN // 128  # 6
TQ = 512             # queries per q-tile
NQ = S // 2          # queries per core
N_QT = NQ // TQ      # 4 q-tiles
SCALE = 1.0 / np.sqrt(np.float32(D))
CSHIFT = 2.0         # exp(s*SCALE - CSHIFT); num/den ratio is C-invariant
AF = mybir.ActivationFunctionType
DR = mybir.MatmulPerfMode.DoubleRow
N_WARM = 3

# ---------------------------------------------------------------------------
# Workarounds: the walrus build in this container accepts only ONE sync-wait
# command per instruction. TileContext's exit drain carries one wait per
# active proc, and Tile's sem assignment emits multi-wait instructions.
# Split both onto single-wait carrier instructions.
# ---------------------------------------------------------------------------


def _split_drain_and_barrier(self, tick_clock, wait_clock):
    gc = tick_clock.global_clock
    engs = [self.nc.sync, self.nc.scalar, self.nc.vector, self.nc.tensor]
    k = 0
    for p in range(N_PROCS):
        if gc[p] == 0:
            continue
        vc = VectorClock([gc[q] if q == p else 0 for q in range(N_PROCS)])
        d = engs[k % len(engs)].drain()
        k += 1
        wait_clock.add_sem_waits(d.ins, ScopedClock({None: vc}))
    self.nc.all_engine_barrier()
    assert self.sems is not None
    popped = self.nc._tile_sem_poison_stack.pop()
    assert popped is self._sem_poison
    self.nc.clear_and_free_semaphores(list(self.sems.allocated().values()))
    self.nc.all_engine_barrier()


tile_mod.TileContext._drain_and_barrier = _split_drain_and_barrier


def _split_waits(nc, max_waits=1):
    for fn in nc.m.functions:
        for bb in fn.blocks:
            insts = bb.instructions
            if not any(
                i.sync_info and i.sync_info.on_wait
                and len(i.sync_info.on_wait) > max_waits
                for i in insts
            ):
                continue
            new = []
            for inst in insts:
                si = inst.sync_info
                ow = list(si.on_wait) if si and si.on_wait else []
                if len(ow) > max_waits:
                    excess, keep = ow[:-max_waits], ow[-max_waits:]
                    for j, w in enumerate(excess):
                        new.append(
                            mybir.InstEventSemaphore(
                                name=f"{inst.name}-wsplit{j}",
                                engine=inst.engine,
                                ins=[],
                                outs=[],
                                sync_info=mybir.SyncInfo(
                                    on_wait=[w], on_update=[]
                                ),
                            )
                        )
                    inst.sync_info = mybir.SyncInfo(
                        on_wait=keep, on_update=list(si.on_update or [])
                    )
                new.append(inst)
            bb.instructions = new


# ---------------------------------------------------------------------------
# Device program
# ---------------------------------------------------------------------------


def _build():
    nc = bass.Bass()
    x8 = nc.declare_dram_parameter("x8", [D_IN, S], f8, isOutput=False)
    xh = nc.declare_dram_parameter("xh", [D_IN, 512], f16, isOutput=False)
    W8 = nc.declare_dram_parameter("W8", [128, 9 * 256], f8, isOutput=False)
    Wh = nc.declare_dram_parameter("Wh", [128, N_DIN * 3 * D], f16,
                                   isOutput=False)
    mask = nc.declare_dram_parameter("mask", [128, 4 * 128], f16,
                                     isOutput=False)
    out_num = nc.declare_dram_parameter("out_num", [D, NQ], f16, isOutput=True)
    out_den = nc.declare_dram_parameter("out_den", [1, NQ], f32, isOutput=True)

    with tile_mod.TileContext(nc) as tc:
        with (
            tc.tile_pool(name="persist", bufs=1) as persist,
            tc.tile_pool(name="work", bufs=6) as work,
            tc.tile_pool(name="outp", bufs=2) as outp,
            tc.tile_pool(name="ps_big", bufs=2, space="PSUM") as ps_big,
            tc.tile_pool(name="ps_out", bufs=1, space="PSUM") as ps_out,
            tc.tile_pool(name="ps_sml", bufs=2, space="PSUM") as ps_sml,
            tc.tile_pool(name="ps_den", bufs=1, space="PSUM") as ps_den,
        ):
            x_all8 = persist.tile([128, N_DIN, S], f8, tag="x_all8")
            xh_all = persist.tile([128, N_DIN, 512], f16, tag="xh_all")
            w8_all = persist.tile([128, 9, 2, 128], f8, tag="w8_all")
            wh_all = persist.tile([128, N_DIN * 3 * D], f16, tag="wh_all")
            m_all = persist.tile([128, 4 * 128], f16, tag="m_all")
            kt_sb = [persist.tile([128, 512], f16, tag=f"kt{c}", name=f"kt{c}")
                     for c in range(S // 512)]
            qt_sb = [persist.tile([128, TQ], f16, tag=f"qt{t}", name=f"qt{t}")
                     for t in range(N_QT)]
            # v8_sb[g][p, j, d] = v[key 128*(4g+j)+p, d] in fp8
            v8_sb = [persist.tile([128, 4, 128], f8, tag=f"v{g}", name=f"v{g}")
                     for g in range(S // 512)]
            # exact f16 v for key blocks 0,1 (own) and 16,17 (other)
            v16h = [persist.tile([128, 2, 128], f16, tag=f"vh{i}",
                                 name=f"vh{i}") for i in range(2)]
            # dedicated diag pt tiles; masked heads zeroed once, then only
            # the live region is ever rewritten -> full-width DR AV/den safe.
            # types: 0=A-own(los 0,128) 1=B-own(256,384) 2=A-oth 3=B-oth
            ptd = [persist.tile([128, 2, TQ], f8, tag=f"ptd{i}",
                                name=f"ptd{i}") for i in range(4)]
            # f16 carve-out pt for t=0 pairs kp=0 / kp=16
            pt16 = [persist.tile([128, 2, TQ], f16, tag=f"pt16_{i}",
                                 name=f"pt16_{i}") for i in range(2)]
            ones8 = persist.tile([128, 2, 128], f8, tag="ones8")
            ones16 = persist.tile([128, 128], f16, tag="ones16")
            biasC = persist.tile([128, 1], f32, tag="biasC")
            warm_sb = persist.tile([128, 1024], f16, tag="warm")

            # W8 host layout: [K|V|Q] blocks, 3 di-pairs each: idx = 3*b+m
            def w8_k(m):
                return w8_all[:, 0 + m]

            def w8_v(m):
                return w8_all[:, 3 + m]

            def w8_q(m):
                return w8_all[:, 6 + m]

            # Wh host layout: [K|V|Q], di-major inside
            def wh_k(di):
                return wh_all[:, 128 * di:128 * (di + 1)]

            def wh_v(di):
                return wh_all[:, 768 + 128 * di:768 + 128 * (di + 1)]

            def wh_q(di):
                return wh_all[:, 1536 + 128 * di:1536 + 128 * (di + 1)]

            tri = [m_all[:, 0:128], m_all[:, 128:256]]  # half1, half2
            ident = m_all[:, 256:384]
            fullm = m_all[:, 384:512]  # all -1000

            # input DMAs. sync: W8 + mask (+ per-tile outputs later);
            # vector: xh + Wh (head projections are the first real PE work);
            # gpsimd: x8 column waves (per-di for wave(0,0)).
            # sync ring (priority FIFO): head + weights first, then the
            # later x8 waves. gpsimd ring: memsets only.
            nc.sync.dma_start(out=xh_all[:], in_=xh.rearrange(
                "(d p) c -> p d c", p=128))
            nc.sync.dma_start(out=wh_all[:], in_=Wh[:])
            nc.sync.dma_start(out=w8_all[:], in_=W8.rearrange(
                "p (i a b) -> p i a b", i=9, a=2))
            nc.sync.dma_start(out=m_all[:], in_=mask[:])
            xsrc = x8.rearrange("(d p) c -> p d c", p=128)
            half = S // 2

            def x_wave(t, h, eng):
                lo = 512 * t + half * h
                eng.dma_start(
                    out=x_all8[:, :, lo:lo + 512], in_=xsrc[:, :, lo:lo + 512]
                )

            nc.gpsimd.memset(warm_sb[:], 0.0)
            for di in range(N_DIN):  # wave(0,0) per-di
                nc.sync.dma_start(
                    out=x_all8[:, di, 0:512], in_=xsrc[:, di, 0:512]
                )
            x_wave(0, 1, nc.sync)
            for t in (1, 2, 3):
                x_wave(t, 0, nc.sync)
                x_wave(t, 1, nc.sync)
            nc.gpsimd.memset(biasC[:], -float(CSHIFT))
            # B-diag pt heads [0:256) stay permanently zero (exp only ever
            # writes [256:512)); A-diag pairs write their full range
            for i in (1, 3):
                nc.gpsimd.memset(ptd[i][:, :, 0:256], 0.0)
            nc.gpsimd.memset(ones8[:], 1.0)
            nc.gpsimd.memset(ones16[:], 1.0)

            # PE pre-warm bridging the input-DMA wait (HAM + pstate
            # ramp). 1024-col matmuls into the (preamble-idle) score psum
            # pool keep duty high despite the 2-buf WAR rotation.
            for i in range(N_WARM):
                psw = ps_big.tile([128, 2, TQ], f32, tag="big",
                                  name=f"warm{i}")
                for s_ in (0, 1):
                    nc.tensor.matmul(
                        psw[:, s_, :], lhsT=warm_sb[:, 0:128],
                        rhs=warm_sb[:, 512 * s_:512 * (s_ + 1)],
                        start=True, stop=True,
                    )

            def x8_cols(m, c0, c1):
                return x_all8[:, 2 * m:2 * m + 2, c0:c1]

            # ---- head (f16-exact) projections: deps only on xh + Wh ----
            def project_heads():
                # kt0[:,0:256], kt4[:,0:256], qt0[:,0:256] and v16h from the
                # f16 head (own rows 0:256 = xh cols 0:256, other = 256:512)
                for dst, wsel, hcol in (
                    (kt_sb[0], wh_k, 0), (kt_sb[4], wh_k, 256),
                    (qt_sb[0], wh_q, 0),
                ):
                    ps = ps_sml.tile([128, 512], f32, tag="sml",
                                     name=f"ph{hcol}_{dst.name}")
                    for di in range(N_DIN):
                        nc.tensor.matmul(
                            ps[:, 0:256], lhsT=wsel(di),
                            rhs=xh_all[:, di, hcol:hcol + 256],
                            start=(di == 0), stop=(di == N_DIN - 1),
                        )
                    nc.vector.tensor_copy(dst[:, 0:256], ps[:, 0:256])
                for i in range(2):  # v16h own/other
                    ps = ps_sml.tile([128, 512], f32, tag="sml",
                                     name=f"phv{i}")
                    for j in range(2):
                        c0 = 256 * i + 128 * j
                        for di in range(N_DIN):
                            nc.tensor.matmul(
                                ps[:, 128 * j:128 * (j + 1)],
                                lhsT=xh_all[:, di, c0:c0 + 128],
                                rhs=wh_v(di),
                                start=(di == 0), stop=(di == N_DIN - 1),
                            )
                    nc.vector.tensor_copy(v16h[i][:], ps[:, 0:256])

            # ---- fp8 DoubleRow projections ----
            def project_kt8(c):
                lo = 256 if c in (0, 4) else 0  # head owns [0:256)
                ps = ps_sml.tile([128, 512], f32, tag="sml", name=f"pkt{c}")
                for m in range(3):
                    nc.tensor.matmul(
                        ps[:, lo:512], lhsT=w8_k(m),
                        rhs=x8_cols(m, 512 * c + lo, 512 * (c + 1)),
                        start=(m == 0), stop=(m == 2), perf_mode=DR,
                    )
                nc.vector.tensor_copy(kt_sb[c][:, lo:512], ps[:, lo:512])

            def project_qt8(t):
                lo = 256 if t == 0 else 0
                ps = ps_sml.tile([128, 512], f32, tag="sml", name=f"pqt{t}")
                for m in range(3):
                    nc.tensor.matmul(
                        ps[:, lo:512], lhsT=w8_q(m),
                        rhs=x8_cols(m, TQ * t + lo, TQ * (t + 1)),
                        start=(m == 0), stop=(m == 2), perf_mode=DR,
                    )
                nc.vector.tensor_copy(qt_sb[t][:, lo:512], ps[:, lo:512])

            def project_v8_group(g):
                ps = ps_sml.tile([128, 512], f32, tag="sml", name=f"pv{g}")
                for j in range(4):
                    k = 4 * g + j
                    for m in range(3):
                        nc.tensor.matmul(
                            ps[:, 128 * j:128 * (j + 1)],
                            lhsT=x8_cols(m, 128 * k, 128 * (k + 1)),
                            rhs=w8_v(m),
                            start=(m == 0), stop=(m == 2), perf_mode=DR,
                        )
                nc.vector.tensor_copy(v8_sb[g][:], ps[:])

            def proj_ops(t):
                """Projection op closures for q-tile t (emitted one tile
                early). For the last tile some projections are deferred into
                its own pair loop (late_ops) as PE filler under the final
                exp chain."""
                if t >= N_QT:
                    return []
                if t == N_QT - 1:
                    return [
                        lambda: project_qt8(t),
                        lambda: project_kt8(t),
                    ]
                return [
                    lambda: project_kt8(t),
                    lambda: project_kt8(N_QT + t),
                    lambda: project_v8_group(t),
                    lambda: project_v8_group(N_QT + t),
                    lambda: project_qt8(t),
                ]

            def fill_ops(t):
                if t == 0:
                    return proj_ops(1)
                return proj_ops(t + 1)

            def late_ops(t):
                if t != N_QT - 1:
                    return []
                return [
                    (3, lambda: project_v8_group(t)),
                    (9, lambda: project_kt8(N_QT + t)),
                    (11, lambda: project_v8_group(N_QT + t)),
                ]

            # ---- preamble projection schedule ----
            project_heads()
            project_kt8(0)
            project_qt8(0)
            project_v8_group(0)
            project_kt8(N_QT)
            project_v8_group(N_QT)

            n_kt_half = NQ // 128  # 16

            for t in range(N_QT):
                pairs = [2 * j for j in range(2 * (t + 1))] + [
                    n_kt_half + 2 * j for j in range(2 * (t + 1))
                ]
                n = len(pairs)

                def pair_info(kp):
                    half2 = kp >= n_kt_half
                    rel = kp - n_kt_half if half2 else kp
                    diag = 4 * t <= rel < 4 * t + 4
                    los = (
                        [128 * (rel - 4 * t), 128 * (rel - 4 * t + 1)]
                        if diag else [0, 0]
                    )
                    carve = t == 0 and rel == 0
                    return half2, diag, los, carve

                def emit_scores(kp, name):
                    # non-carve diag pairs: both s_ cover [lo0:512) so exp is
                    # a single 3D slice; s_=1's extra 128 cols get a full
                    # -1000 mask (exp -> 0). carve keeps per-s_ ranges.
                    half2, diag, los, carve = pair_info(kp)
                    ps = ps_big.tile([128, 2, TQ], f32, tag="big", name=name)
                    for s_ in (0, 1):
                        kt = kp + s_
                        lo = los[s_] if (carve or not diag) else los[0]
                        nc.tensor.matmul(
                            ps[:, s_, lo:TQ],
                            lhsT=kt_sb[kt // 4][:, 128 * (kt % 4):128 * (kt % 4 + 1)],
                            rhs=qt_sb[t][:, lo:TQ],
                            start=True,
                            stop=not diag,
                            skip_group_check=diag,
                        )
                        if diag:
                            if s_ == 1 and not carve:
                                nc.tensor.matmul(
                                    ps[:, s_, lo:lo + 128],
                                    lhsT=ident,
                                    rhs=fullm,
                                    start=False,
                                    stop=False,
                                    skip_group_check=True,
                                )
                            nc.tensor.matmul(
                                ps[:, s_, los[s_]:los[s_] + 128],
                                lhsT=ident,
                                rhs=tri[1 if half2 else 0],
                                start=False,
                                stop=True,
                                skip_group_check=True,
                            )
                    return ps

                # software pipeline: scores one pair ahead of exp/AV
                fill = fill_ops(t)
                fill_done = 0
                late = list(late_ops(t))

                po = ps_out.tile([128, TQ], f32, tag="out", name=f"po{t}")
                dn = ps_den.tile([128, TQ], f32, tag="den", name=f"dn{t}")

                fa = [True]
                fd = [True]

                def emit_avden(kp, pt, last):
                    half2, diag, los, carve = pair_info(kp)
                    g, j = kp // 4, kp % 4
                    if carve:
                        vh = v16h[1 if half2 else 0]
                        for s_ in (0, 1):
                            lo = los[s_]
                            nc.tensor.matmul(
                                po[:, lo:TQ], lhsT=vh[:, s_, :],
                                rhs=pt[:, s_, lo:TQ],
                                start=fa[0], stop=False,
                                skip_group_check=True,
                            )
                            fa[0] = False
                            nc.tensor.matmul(
                                dn[:, lo:TQ], lhsT=ones16[:],
                                rhs=pt[:, s_, lo:TQ],
                                start=fd[0], stop=False,
                                skip_group_check=True,
                            )
                            fd[0] = False
                    else:
                        nc.tensor.matmul(
                            po[:], lhsT=v8_sb[g][:, j:j + 2, :], rhs=pt[:],
                            start=fa[0], stop=last,
                            perf_mode=DR, skip_group_check=True,
                        )
                        fa[0] = False
                        nc.tensor.matmul(
                            dn[:], lhsT=ones8[:], rhs=pt[:],
                            start=fd[0], stop=last,
                            perf_mode=DR, skip_group_check=True,
                        )
                        fd[0] = False

                def emit_exp(kp, ps, name):
                    half2, diag, los, carve = pair_info(kp)
                    if carve:
                        pt = pt16[1 if half2 else 0]
                        for s_ in (0, 1):
                            lo = los[s_]
                            nc.scalar.activation(
                                pt[:, s_, lo:TQ], ps[:, s_, lo:TQ],
                                AF.Exp, scale=float(SCALE), bias=biasC[:],
                            )
                    elif diag and los[0]:
                        # B-diag: dedicated tile, head [0:256) stays 0
                        pt = ptd[3 if half2 else 1]
                        lo0 = los[0]
                        nc.scalar.activation(
                            pt[:, :, lo0:TQ], ps[:, :, lo0:TQ],
                            AF.Exp, scale=float(SCALE), bias=biasC[:],
                        )
                    else:
                        # off-diag and A-diag: full-width single exp
                        pt = work.tile([128, 2, TQ], f8, tag="pt", name=name)
                        nc.scalar.activation(
                            pt[:], ps[:], AF.Exp, scale=float(SCALE),
                            bias=biasC[:],
                        )
                    return pt

                while late and late[0][0] <= 0:
                    late.pop(0)[1]()
                ps_q = [emit_scores(pairs[0], f"s{t}_0")]
                pend = []  # (kp, pt) with AV/den deferred by one step
                for i, kp in enumerate(pairs):
                    if i + 1 < n:
                        ps_q.append(emit_scores(pairs[i + 1], f"s{t}_{i + 1}"))
                    # AV/den run one pair behind their exp: hides the exp
                    # latency and the po/dn WAR at tile boundaries
                    if pend:
                        pkp, ppt = pend.pop(0)
                        emit_avden(pkp, ppt, False)
                    want = ((i + 1) * len(fill)) // n
                    while fill_done < want:
                        fill[fill_done]()
                        fill_done += 1
                    while late and late[0][0] <= i + 1:
                        late.pop(0)[1]()
                    ps = ps_q.pop(0)
                    pt = emit_exp(kp, ps, f"p{t}_{kp}")
                    pend.append((kp, pt))
                pkp, ppt = pend.pop(0)
                emit_avden(pkp, ppt, True)
                ob = outp.tile([128, TQ], f16, tag="ob", name=f"ob{t}")
                db = outp.tile([1, TQ], f32, tag="db", name=f"db{t}")
                nc.vector.tensor_copy(db[:], dn[0:1, :])
                nc.vector.tensor_copy(ob[:], po[:])
                nc.sync.dma_start(
                    out=out_den[:, TQ * t:TQ * (t + 1)], in_=db[:]
                )
                nc.sync.dma_start(out=out_num[:, TQ * t:TQ * (t + 1)], in_=ob[:])
    _split_waits(nc)
    return nc


_NC_CACHE = []


def _get_nc():
    if not _NC_CACHE:
        _NC_CACHE.append(_build())
    return _NC_CACHE[0]


def _host_inputs(x, Wq, Wk, Wv):
    # W8 layout: [K|V|Q] blocks, each [128, 3 pairs, 2, 128] -> [128, 2304]
    def blk8(M):
        return (M.astype(np.float32).reshape(3, 2, 128, D)
                .transpose(2, 0, 1, 3).reshape(128, 768))

    W8 = np.ascontiguousarray(
        np.concatenate([blk8(Wk), blk8(Wv), blk8(Wq)], axis=1)
    ).astype(E4)

    # Wh layout: [K|V|Q], di-major inside (f16)
    def blkh(M):
        return M.astype(np.float16).reshape(N_DIN, 128, D).transpose(1, 0, 2)

    Wh = np.ascontiguousarray(
        np.concatenate([blkh(Wk), blkh(Wv), blkh(Wq)], axis=1)
        .reshape(128, N_DIN * 3 * D)
    )
    u = np.arange(128)[:, None]
    i = np.arange(128)[None, :]
    masks = {}
    for h in (0, 1):
        tri1 = (u <= i).astype(np.float32)          # own-parity half
        tri2 = (u <= i - 1 + h).astype(np.float32)  # other-parity half
        ma = np.concatenate(
            [(tri1 - 1.0) * 1000.0, (tri2 - 1.0) * 1000.0,
             np.eye(128, dtype=np.float32),
             np.full((128, 128), -1000.0, dtype=np.float32)], axis=1
        )
        masks[h] = np.ascontiguousarray(ma).astype(np.float16)
    in_maps = []
    for c in range(2 * B):
        b, h = divmod(c, 2)
        xp = np.concatenate([x[b, h::2], x[b, 1 - h::2]], axis=0)  # [S, 768]
        x8_p = np.ascontiguousarray(xp.T).astype(E4)  # [768, S]
        xh_p = np.ascontiguousarray(
            np.concatenate([xp[0:256], xp[2048:2304]], axis=0).T
        ).astype(np.float16)  # [768, 512]
        in_maps.append({"x8": x8_p, "xh": xh_p, "W8": W8, "Wh": Wh,
                        "mask": masks[h]})
    return in_maps


def kernel(x, Wq, Wk, Wv):
    x = np.asarray(x, np.float32)
    Wq = np.asarray(Wq, np.float32)
    Wk = np.asarray(Wk, np.float32)
    Wv = np.asarray(Wv, np.float32)
    nc = _get_nc()
    in_maps = _host_inputs(x, Wq, Wk, Wv)
    res = run_bass_kernel_spmd(nc, in_maps, list(range(2 * B)))
    out = np.empty((B, S, D), np.float32)
    for c in range(2 * B):
        b, h = divmod(c, 2)
        num = res.results[c]["out_num"].astype(np.float32)  # [128, NQ]
        den = res.results[c]["out_den"][0]       # [NQ] f32
        out[b, h::2, :] = (num / den[None, :]).T
    return out


# revision 16
# speedup vs baseline: 1.0854x; 1.0010x over previous
"""Causal-attention (QKV projection + softmax(QK^T/sqrt(d))V) on 8 trn2 cores.

Contract: kernel(x, Wq, Wk, Wv) takes FULL inputs
  x [4, 4096, 768] f32, Wq/Wk/Wv [768, 128] f32
and returns the FULL output [4, 4096, 128] f32.

Sharding: 2 cores per batch. Core with parity h in {0,1} of batch b owns query
rows h::2 (perfect causal load balance). The host permutes the per-core input
to xT_p = concat(x[b, h::2], x[b, 1-h::2]).T so one compiled SPMD program runs
on every core; causality is enforced with per-core [128,128] triangular
additive-mask tiles applied only on the diagonal 128-key blocks.

v3 changes vs v2 (83.4us):
  - x shipped fp8e4 (half the input bytes) + a small f16 "head" (first 256
    rows of each parity) for exact early-row projections
  - QKV projections in fp8 DoubleRow (2 k-tiles per pass)
  - P = exp(s*SCALE - C) written fp8e4 (C=2.0 keeps max p ~103 < 240);
    AV matmul in fp8 DoubleRow (both key blocks of a pair in one matmul)
  - denominator via ones-weights matmuls on the PE accumulating into a
    [16,512] psum tile per q-tile (replaces the DVE sacc accumulation and
    the 1MB out_den DMA; host divide now reads a [1,2048] f32 den)
  - diagonal pairs use 4 dedicated pt tiles whose masked head regions are
    zeroed once and never rewritten, so AV/den run full-width DoubleRow
  - f16 carve-out for key blocks 0,1/16,17 vs queries [0,256): head-exact
    q/k/v + f16 pt + plain-mode AV/den protect all rows with <256 keys
"""
import numpy as np
import ml_dtypes

import concourse.bass as bass
import concourse.mybir as mybir
import concourse.tile as tile_mod
from concourse.tile import ScopedClock, VectorClock
from concourse.tile_sem_assignment import N_PROCS
from concourse.bass_utils import run_bass_kernel_spmd

f32 = mybir.dt.float32
f16 = mybir.dt.float16
f8 = mybir.dt.float8e4
E4 = ml_dtypes.float8_e4m3

B, S, D_IN, D = 4, 4096, 768, 128
N_DIN = D_IN // 128  # 6
TQ = 512             # queries per q-tile
NQ = S // 2          # queries per core
N_QT = NQ // TQ      # 4 q-tiles
SCALE = 1.0 / np.sqrt(np.float32(D))
CSHIFT = 2.0         # exp(s*SCALE - CSHIFT); num/den ratio is C-invariant
AF = mybir.ActivationFunctionType
DR = mybir.MatmulPerfMode.DoubleRow
N_WARM = 3

# ---------------------------------------------------------------------------
# Workarounds: the walrus build in this container accepts only ONE sync-wait
# command per instruction. TileContext's exit drain carries one wait per
# active proc, and Tile's sem assignment emits multi-wait instructions.
# Split both onto single-wait carrier instructions.
# ---------------------------------------------------------------------------


def _split_drain_and_barrier(self, tick_clock, wait_clock):
    gc = tick_clock.global_clock
    engs = [self.nc.sync, self.nc.scalar, self.nc.vector, self.nc.tensor]
    k = 0
    for p in range(N_PROCS):
        if gc[p] == 0:
            continue
        vc = VectorClock([gc[q] if q == p else 0 for q in range(N_PROCS)])
        d = engs[k % len(engs)].drain()
        k += 1
        wait_clock.add_sem_waits(d.ins, ScopedClock({None: vc}))
    self.nc.all_engine_barrier()
    assert self.sems is not None
    popped = self.nc._tile_sem_poison_stack.pop()
    assert popped is self._sem_poison
    self.nc.clear_and_free_semaphores(list(self.sems.allocated().values()))
    self.nc.all_engine_barrier()


tile_mod.TileContext._drain_and_barrier = _split_drain_and_barrier


def _split_waits(nc, max_waits=1):
    for fn in nc.m.functions:
        for bb in fn.blocks:
            insts = bb.instructions
            if not any(
                i.sync_info and i.sync_info.on_wait
                and len(i.sync_info.on_wait) > max_waits
                for i in insts
            ):
                continue
            new = []
            for inst in insts:
                si = inst.sync_info
                ow = list(si.on_wait) if si and si.on_wait else []
                if len(ow) > max_waits:
                    excess, keep = ow[:-max_waits], ow[-max_waits:]
                    for j, w in enumerate(excess):
                        new.append(
                            mybir.InstEventSemaphore(
                                name=f"{inst.name}-wsplit{j}",
                                engine=inst.engine,
                                ins=[],
                                outs=[],
                                sync_info=mybir.SyncInfo(
                                    on_wait=[w], on_update=[]
                                ),
                            )
                        )
                    inst.sync_info = mybir.SyncInfo(
                        on_wait=keep, on_update=list(si.on_update or [])
                    )
                new.append(inst)
            bb.instructions = new


# ---------------------------------------------------------------------------
# Device program
# ---------------------------------------------------------------------------


def _build():
    nc = bass.Bass()
    x8 = nc.declare_dram_parameter("x8", [D_IN, S], f8, isOutput=False)
    xh = nc.declare_dram_parameter("xh", [D_IN, 512], f16, isOutput=False)
    W8 = nc.declare_dram_parameter("W8", [128, 9 * 256], f8, isOutput=False)
    Wh = nc.declare_dram_parameter("Wh", [128, N_DIN * 3 * D], f16,
                                   isOutput=False)
    mask = nc.declare_dram_parameter("mask", [128, 4 * 128], f16,
                                     isOutput=False)
    out_num = nc.declare_dram_parameter("out_num", [D, NQ], f16, isOutput=True)
    out_den = nc.declare_dram_parameter("out_den", [1, NQ], f32, isOutput=True)

    with tile_mod.TileContext(nc) as tc:
        with (
            tc.tile_pool(name="persist", bufs=1) as persist,
            tc.tile_pool(name="work", bufs=6) as work,
            tc.tile_pool(name="outp", bufs=2) as outp,
            tc.tile_pool(name="ps_big", bufs=2, space="PSUM") as ps_big,
            tc.tile_pool(name="ps_out", bufs=1, space="PSUM") as ps_out,
            tc.tile_pool(name="ps_sml", bufs=2, space="PSUM") as ps_sml,
            tc.tile_pool(name="ps_den", bufs=1, space="PSUM") as ps_den,
        ):
            x_all8 = persist.tile([128, N_DIN, S], f8, tag="x_all8")
            xh_all = persist.tile([128, N_DIN, 512], f16, tag="xh_all")
            w8_all = persist.tile([128, 9, 2, 128], f8, tag="w8_all")
            wh_all = persist.tile([128, N_DIN * 3 * D], f16, tag="wh_all")
            m_all = persist.tile([128, 4 * 128], f16, tag="m_all")
            kt_sb = [persist.tile([128, 512], f16, tag=f"kt{c}", name=f"kt{c}")
                     for c in range(S // 512)]
            qt_sb = [persist.tile([128, TQ], f16, tag=f"qt{t}", name=f"qt{t}")
                     for t in range(N_QT)]
            # v8_sb[g][p, j, d] = v[key 128*(4g+j)+p, d] in fp8
            v8_sb = [persist.tile([128, 4, 128], f8, tag=f"v{g}", name=f"v{g}")
                     for g in range(S // 512)]
            # exact f16 v for key blocks 0,1 (own) and 16,17 (other)
            v16h = [persist.tile([128, 2, 128], f16, tag=f"vh{i}",
                                 name=f"vh{i}") for i in range(2)]
            # dedicated diag pt tiles; masked heads zeroed once, then only
            # the live region is ever rewritten -> full-width DR AV/den safe.
            # types: 0=A-own(los 0,128) 1=B-own(256,384) 2=A-oth 3=B-oth
            ptd = [persist.tile([128, 2, TQ], f8, tag=f"ptd{i}",
                                name=f"ptd{i}") for i in range(4)]
            # f16 carve-out pt for t=0 pairs kp=0 / kp=16
            pt16 = [persist.tile([128, 2, TQ], f16, tag=f"pt16_{i}",
                                 name=f"pt16_{i}") for i in range(2)]
            ones8 = persist.tile([128, 2, 128], f8, tag="ones8")
            ones16 = persist.tile([128, 128], f16, tag="ones16")
            biasC = persist.tile([128, 1], f32, tag="biasC")
            warm_sb = persist.tile([128, 1024], f16, tag="warm")

            # W8 host layout: [K|V|Q] blocks, 3 di-pairs each: idx = 3*b+m
            def w8_k(m):
                return w8_all[:, 0 + m]

            def w8_v(m):
                return w8_all[:, 3 + m]

            def w8_q(m):
                return w8_all[:, 6 + m]

            # Wh host layout: [K|V|Q], di-major inside
            def wh_k(di):
                return wh_all[:, 128 * di:128 * (di + 1)]

            def wh_v(di):
                return wh_all[:, 768 + 128 * di:768 + 128 * (di + 1)]

            def wh_q(di):
                return wh_all[:, 1536 + 128 * di:1536 + 128 * (di + 1)]

            tri = [m_all[:, 0:128], m_all[:, 128:256]]  # half1, half2
            ident = m_all[:, 256:384]
            fullm = m_all[:, 384:512]  # all -1000

            # input DMAs. sync: W8 + mask (+ per-tile outputs later);
            # vector: xh + Wh (head projections are the first real PE work);
            # gpsimd: x8 column waves (per-di for wave(0,0)).
            # sync ring (priority FIFO): head + weights first, then the
            # later x8 waves. gpsimd ring: memsets only.
            nc.sync.dma_start(out=xh_all[:], in_=xh.rearrange(
                "(d p) c -> p d c", p=128))
            nc.sync.dma_start(out=wh_all[:], in_=Wh[:])
            nc.sync.dma_start(out=w8_all[:], in_=W8.rearrange(
                "p (i a b) -> p i a b", i=9, a=2))
            nc.sync.dma_start(out=m_all[:], in_=mask[:])
            xsrc = x8.rearrange("(d p) c -> p d c", p=128)
            half = S // 2

            def x_wave(t, h, eng):
                lo = 512 * t + half * h
                eng.dma_start(
                    out=x_all8[:, :, lo:lo + 512], in_=xsrc[:, :, lo:lo + 512]
                )

            nc.gpsimd.memset(warm_sb[:], 0.0)
            for di in range(N_DIN):  # wave(0,0) per-di
                nc.sync.dma_start(
                    out=x_all8[:, di, 0:512], in_=xsrc[:, di, 0:512]
                )
            x_wave(0, 1, nc.sync)
            for t in (1, 2, 3):
                x_wave(t, 0, nc.sync)
                x_wave(t, 1, nc.sync)
            nc.gpsimd.memset(biasC[:], -float(CSHIFT))
            # B-diag pt heads [0:256) stay permanently zero (exp only ever
            # writes [256:512)); A-diag pairs write their full range
            for i in (1, 3):
                nc.gpsimd.memset(ptd[i][:, :, 0:256], 0.0)
            nc.gpsimd.memset(ones8[:], 1.0)
            nc.gpsimd.memset(ones16[:], 1.0)

            # PE pre-warm bridging the input-DMA wait (HAM + pstate
            # ramp). 1024-col matmuls into the (preamble-idle) score psum
            # pool keep duty high despite the 2-buf WAR rotation.
            for i in range(N_WARM):
                psw = ps_big.tile([128, 2, TQ], f32, tag="big",
                                  name=f"warm{i}")
                for s_ in (0, 1):
                    nc.tensor.matmul(
                        psw[:, s_, :], lhsT=warm_sb[:, 0:128],
                        rhs=warm_sb[:, 512 * s_:512 * (s_ + 1)],
                        start=True, stop=True,
                    )

            def x8_cols(m, c0, c1):
                return x_all8[:, 2 * m:2 * m + 2, c0:c1]

            # ---- head (f16-exact) projections: deps only on xh + Wh ----
            def project_heads():
                # kt0[:,0:256], kt4[:,0:256], qt0[:,0:256] and v16h from the
                # f16 head (own rows 0:256 = xh cols 0:256, other = 256:512)
                for dst, wsel, hcol in (
                    (kt_sb[0], wh_k, 0), (kt_sb[4], wh_k, 256),
                    (qt_sb[0], wh_q, 0),
                ):
                    ps = ps_sml.tile([128, 512], f32, tag="sml",
                                     name=f"ph{hcol}_{dst.name}")
                    for di in range(N_DIN):
                        nc.tensor.matmul(
                            ps[:, 0:256], lhsT=wsel(di),
                            rhs=xh_all[:, di, hcol:hcol + 256],
                            start=(di == 0), stop=(di == N_DIN - 1),
                        )
                    nc.vector.tensor_copy(dst[:, 0:256], ps[:, 0:256])
                for i in range(2):  # v16h own/other
                    ps = ps_sml.tile([128, 512], f32, tag="sml",
                                     name=f"phv{i}")
                    for j in range(2):
                        c0 = 256 * i + 128 * j
                        for di in range(N_DIN):
                            nc.tensor.matmul(
                                ps[:, 128 * j:128 * (j + 1)],
                                lhsT=xh_all[:, di, c0:c0 + 128],
                                rhs=wh_v(di),
                                start=(di == 0), stop=(di == N_DIN - 1),
                            )
                    nc.vector.tensor_copy(v16h[i][:], ps[:, 0:256])

            # ---- fp8 DoubleRow projections ----
            def project_kt8(c):
                lo = 256 if c in (0, 4) else 0  # head owns [0:256)
                ps = ps_sml.tile([128, 512], f32, tag="sml", name=f"pkt{c}")
                for m in range(3):
                    nc.tensor.matmul(
                        ps[:, lo:512], lhsT=w8_k(m),
                        rhs=x8_cols(m, 512 * c + lo, 512 * (c + 1)),
                        start=(m == 0), stop=(m == 2), perf_mode=DR,
                    )
                nc.vector.tensor_copy(kt_sb[c][:, lo:512], ps[:, lo:512])

            def project_qt8(t):
                lo = 256 if t == 0 else 0
                ps = ps_sml.tile([128, 512], f32, tag="sml", name=f"pqt{t}")
                for m in range(3):
                    nc.tensor.matmul(
                        ps[:, lo:512], lhsT=w8_q(m),
                        rhs=x8_cols(m, TQ * t + lo, TQ * (t + 1)),
                        start=(m == 0), stop=(m == 2), perf_mode=DR,
                    )
                nc.vector.tensor_copy(qt_sb[t][:, lo:512], ps[:, lo:512])

            def project_v8_group(g):
                ps = ps_sml.tile([128, 512], f32, tag="sml", name=f"pv{g}")
                for j in range(4):
                    k = 4 * g + j
                    for m in range(3):
                        nc.tensor.matmul(
                            ps[:, 128 * j:128 * (j + 1)],
                            lhsT=x8_cols(m, 128 * k, 128 * (k + 1)),
                            rhs=w8_v(m),
                            start=(m == 0), stop=(m == 2), perf_mode=DR,
                        )
                nc.vector.tensor_copy(v8_sb[g][:], ps[:])

            def proj_ops(t):
                """Projection op closures for q-tile t (emitted one tile
                early). For the last tile some projections are deferred into
                its own pair loop (late_ops) as PE filler under the final
                exp chain."""
                if t >= N_QT:
                    return []
                if t == N_QT - 1:
                    return [
                        lambda: project_qt8(t),
                        lambda: project_kt8(t),
                    ]
                return [
                    lambda: project_kt8(t),
                    lambda: project_kt8(N_QT + t),
                    lambda: project_v8_group(t),
                    lambda: project_v8_group(N_QT + t),
                    lambda: project_qt8(t),
                ]

            def fill_ops(t):
                if t == 0:
                    return proj_ops(1)
                return proj_ops(t + 1)

            def late_ops(t):
                if t != N_QT - 1:
                    return []
                return [
                    (3, lambda: project_v8_group(t)),
                    (9, lambda: project_kt8(N_QT + t)),
                    (11, lambda: project_v8_group(N_QT + t)),
                ]

            # ---- preamble projection schedule ----
            project_heads()
            project_kt8(0)
            project_qt8(0)
            project_v8_group(0)
            project_kt8(N_QT)
            project_v8_group(N_QT)

            n_kt_half = NQ // 128  # 16

            for t in range(N_QT):
                pairs = [2 * j for j in range(2 * (t + 1))] + [
                    n_kt_half + 2 * j for j in range(2 * (t + 1))
                ]
                n = len(pairs)

                def pair_info(kp):
                    half2 = kp >= n_kt_half
                    rel = kp - n_kt_half if half2 else kp
                    diag = 4 * t <= rel < 4 * t + 4
                    los = (
                        [128 * (rel - 4 * t), 128 * (rel - 4 * t + 1)]
                        if diag else [0, 0]
                    )
                    carve = t == 0 and rel == 0
                    return half2, diag, los, carve

                def emit_scores(kp, name):
                    # non-carve diag pairs: both s_ cover [lo0:512) so exp is
                    # a single 3D slice; s_=1's extra 128 cols get a full
                    # -1000 mask (exp -> 0). carve keeps per-s_ ranges.
                    half2, diag, los, carve = pair_info(kp)
                    ps = ps_big.tile([128, 2, TQ], f32, tag="big", name=name)
                    for s_ in (0, 1):
                        kt = kp + s_
                        lo = los[s_] if (carve or not diag) else los[0]
                        nc.tensor.matmul(
                            ps[:, s_, lo:TQ],
                            lhsT=kt_sb[kt // 4][:, 128 * (kt % 4):128 * (kt % 4 + 1)],
                            rhs=qt_sb[t][:, lo:TQ],
                            start=True,
                            stop=not diag,
                            skip_group_check=diag,
                        )
                        if diag:
                            if s_ == 1 and not carve:
                                nc.tensor.matmul(
                                    ps[:, s_, lo:lo + 128],
                                    lhsT=ident,
                                    rhs=fullm,
                                    start=False,
                                    stop=False,
                                    skip_group_check=True,
                                )
                            nc.tensor.matmul(
                                ps[:, s_, los[s_]:los[s_] + 128],
                                lhsT=ident,
                                rhs=tri[1 if half2 else 0],
                                start=False,
                                stop=True,
                                skip_group_check=True,
                            )
                    return ps

                # software pipeline: scores one pair ahead of exp/AV
                fill = fill_ops(t)
                fill_done = 0
                late = list(late_ops(t))

                po = ps_out.tile([128, TQ], f32, tag="out", name=f"po{t}")
                dn = ps_den.tile([128, TQ], f32, tag="den", name=f"dn{t}")

                fa = [True]
                fd = [True]
                sacc_on = [False]
                odd_ctr = [0]
                sacc = outp.tile([128, TQ], f16, tag="sacc", name=f"sacc{t}")

                def emit_avden(kp, pt, last):
                    half2, diag, los, carve = pair_info(kp)
                    g, j = kp // 4, kp % 4
                    if carve:
                        vh = v16h[1 if half2 else 0]
                        for s_ in (0, 1):
                            lo = los[s_]
                            nc.tensor.matmul(
                                po[:, lo:TQ], lhsT=vh[:, s_, :],
                                rhs=pt[:, s_, lo:TQ],
                                start=fa[0], stop=False,
                                skip_group_check=True,
                            )
                            fa[0] = False
                            nc.tensor.matmul(
                                dn[:, lo:TQ], lhsT=ones16[:],
                                rhs=pt[:, s_, lo:TQ],
                                start=fd[0], stop=False,
                                skip_group_check=True,
                            )
                            fd[0] = False
                    else:
                        nc.tensor.matmul(
                            po[:], lhsT=v8_sb[g][:, j:j + 2, :], rhs=pt[:],
                            start=fa[0], stop=last,
                            perf_mode=DR, skip_group_check=True,
                        )
                        fa[0] = False
                        take_dve = False
                        if not diag:
                            take_dve = odd_ctr[0] % 2 == 1
                            odd_ctr[0] += 1
                        if take_dve:
                            # every 2nd off-diag pair: den partial sums on
                            # the (lightly loaded) DVE, folded into dn once
                            # per tile by a single f16 matmul
                            if not sacc_on[0]:
                                nc.vector.tensor_copy(sacc[:], pt[:, 0, :])
                                sacc_on[0] = True
                            else:
                                nc.vector.tensor_add(
                                    sacc[:], sacc[:], pt[:, 0, :])
                            nc.vector.tensor_add(sacc[:], sacc[:], pt[:, 1, :])
                        else:
                            nc.tensor.matmul(
                                dn[:], lhsT=ones8[:], rhs=pt[:],
                                start=fd[0], stop=last and not sacc_on[0],
                                perf_mode=DR, skip_group_check=True,
                            )
                            fd[0] = False

                def emit_exp(kp, ps, name):
                    half2, diag, los, carve = pair_info(kp)
                    if carve:
                        pt = pt16[1 if half2 else 0]
                        for s_ in (0, 1):
                            lo = los[s_]
                            nc.scalar.activation(
                                pt[:, s_, lo:TQ], ps[:, s_, lo:TQ],
                                AF.Exp, scale=float(SCALE), bias=biasC[:],
                            )
                    elif diag and los[0]:
                        # B-diag: dedicated tile, head [0:256) stays 0
                        pt = ptd[3 if half2 else 1]
                        lo0 = los[0]
                        nc.scalar.activation(
                            pt[:, :, lo0:TQ], ps[:, :, lo0:TQ],
                            AF.Exp, scale=float(SCALE), bias=biasC[:],
                        )
                    else:
                        # off-diag and A-diag: full-width single exp
                        pt = work.tile([128, 2, TQ], f8, tag="pt", name=name)
                        nc.scalar.activation(
                            pt[:], ps[:], AF.Exp, scale=float(SCALE),
                            bias=biasC[:],
                        )
                    return pt

                while late and late[0][0] <= 0:
                    late.pop(0)[1]()
                ps_q = [emit_scores(pairs[0], f"s{t}_0")]
                pend = []  # (kp, pt) with AV/den deferred by one step
                for i, kp in enumerate(pairs):
                    if i + 1 < n:
                        ps_q.append(emit_scores(pairs[i + 1], f"s{t}_{i + 1}"))
                    # AV/den run one pair behind their exp: hides the exp
                    # latency and the po/dn WAR at tile boundaries
                    if pend:
                        pkp, ppt = pend.pop(0)
                        emit_avden(pkp, ppt, False)
                    want = ((i + 1) * len(fill)) // n
                    while fill_done < want:
                        fill[fill_done]()
                        fill_done += 1
                    while late and late[0][0] <= i + 1:
                        late.pop(0)[1]()
                    ps = ps_q.pop(0)
                    pt = emit_exp(kp, ps, f"p{t}_{kp}")
                    pend.append((kp, pt))
                pkp, ppt = pend.pop(0)
                emit_avden(pkp, ppt, True)
                if sacc_on[0]:
                    nc.tensor.matmul(
                        dn[:], lhsT=ones16[:], rhs=sacc[:],
                        start=False, stop=True, skip_group_check=True,
                    )
                ob = outp.tile([128, TQ], f16, tag="ob", name=f"ob{t}")
                db = outp.tile([1, TQ], f32, tag="db", name=f"db{t}")
                nc.vector.tensor_copy(db[:], dn[0:1, :])
                nc.vector.tensor_copy(ob[:], po[:])
                nc.sync.dma_start(
                    out=out_den[:, TQ * t:TQ * (t + 1)], in_=db[:]
                )
                nc.sync.dma_start(out=out_num[:, TQ * t:TQ * (t + 1)], in_=ob[:])
    _split_waits(nc)
    return nc


_NC_CACHE = []


def _get_nc():
    if not _NC_CACHE:
        _NC_CACHE.append(_build())
    return _NC_CACHE[0]


def _host_inputs(x, Wq, Wk, Wv):
    # W8 layout: [K|V|Q] blocks, each [128, 3 pairs, 2, 128] -> [128, 2304]
    def blk8(M):
        return (M.astype(np.float32).reshape(3, 2, 128, D)
                .transpose(2, 0, 1, 3).reshape(128, 768))

    W8 = np.ascontiguousarray(
        np.concatenate([blk8(Wk), blk8(Wv), blk8(Wq)], axis=1)
    ).astype(E4)

    # Wh layout: [K|V|Q], di-major inside (f16)
    def blkh(M):
        return M.astype(np.float16).reshape(N_DIN, 128, D).transpose(1, 0, 2)

    Wh = np.ascontiguousarray(
        np.concatenate([blkh(Wk), blkh(Wv), blkh(Wq)], axis=1)
        .reshape(128, N_DIN * 3 * D)
    )
    u = np.arange(128)[:, None]
    i = np.arange(128)[None, :]
    masks = {}
    for h in (0, 1):
        tri1 = (u <= i).astype(np.float32)          # own-parity half
        tri2 = (u <= i - 1 + h).astype(np.float32)  # other-parity half
        ma = np.concatenate(
            [(tri1 - 1.0) * 1000.0, (tri2 - 1.0) * 1000.0,
             np.eye(128, dtype=np.float32),
             np.full((128, 128), -1000.0, dtype=np.float32)], axis=1
        )
        masks[h] = np.ascontiguousarray(ma).astype(np.float16)
    in_maps = []
    for c in range(2 * B):
        b, h = divmod(c, 2)
        xp = np.concatenate([x[b, h::2], x[b, 1 - h::2]], axis=0)  # [S, 768]
        x8_p = np.ascontiguousarray(xp.T).astype(E4)  # [768, S]
        xh_p = np.ascontiguousarray(
            np.concatenate([xp[0:256], xp[2048:2304]], axis=0).T
        ).astype(np.float16)  # [768, 512]
        in_maps.append({"x8": x8_p, "xh": xh_p, "W8": W8, "Wh": Wh,
                        "mask": masks[h]})
    return in_maps


def kernel(x, Wq, Wk, Wv):
    x = np.asarray(x, np.float32)
    Wq = np.asarray(Wq, np.float32)
    Wk = np.asarray(Wk, np.float32)
    Wv = np.asarray(Wv, np.float32)
    nc = _get_nc()
    in_maps = _host_inputs(x, Wq, Wk, Wv)
    res = run_bass_kernel_spmd(nc, in_maps, list(range(2 * B)))
    out = np.empty((B, S, D), np.float32)
    for c in range(2 * B):
        b, h = divmod(c, 2)
        num = res.results[c]["out_num"].astype(np.float32)  # [128, NQ]
        den = res.results[c]["out_den"][0]       # [NQ] f32
        out[b, h::2, :] = (num / den[None, :]).T
    return out


# revision 17
# speedup vs baseline: 1.1136x; 1.0260x over previous
"""Causal-attention (QKV projection + softmax(QK^T/sqrt(d))V) on 8 trn2 cores.

Contract: kernel(x, Wq, Wk, Wv) takes FULL inputs
  x [4, 4096, 768] f32, Wq/Wk/Wv [768, 128] f32
and returns the FULL output [4, 4096, 128] f32.

Sharding: 2 cores per batch. Core with parity h in {0,1} of batch b owns query
rows h::2 (perfect causal load balance). The host permutes the per-core input
to xT_p = concat(x[b, h::2], x[b, 1-h::2]).T so one compiled SPMD program runs
on every core; causality is enforced with per-core [128,128] triangular
additive-mask tiles applied only on the diagonal 128-key blocks.

v3 changes vs v2 (83.4us):
  - x shipped fp8e4 (half the input bytes) + a small f16 "head" (first 256
    rows of each parity) for exact early-row projections
  - QKV projections in fp8 DoubleRow (2 k-tiles per pass)
  - P = exp(s*SCALE - C) written fp8e4 (C=2.0 keeps max p ~103 < 240);
    AV matmul in fp8 DoubleRow (both key blocks of a pair in one matmul)
  - denominator via ones-weights matmuls on the PE accumulating into a
    [16,512] psum tile per q-tile (replaces the DVE sacc accumulation and
    the 1MB out_den DMA; host divide now reads a [1,2048] f32 den)
  - diagonal pairs use 4 dedicated pt tiles whose masked head regions are
    zeroed once and never rewritten, so AV/den run full-width DoubleRow
  - f16 carve-out for key blocks 0,1/16,17 vs queries [0,256): head-exact
    q/k/v + f16 pt + plain-mode AV/den protect all rows with <256 keys
"""
import numpy as np
import ml_dtypes

import concourse.bass as bass
import concourse.mybir as mybir
import concourse.tile as tile_mod
from concourse.tile import ScopedClock, VectorClock
from concourse.tile_sem_assignment import N_PROCS
from concourse.bass_utils import run_bass_kernel_spmd

f32 = mybir.dt.float32
f16 = mybir.dt.float16
f8 = mybir.dt.float8e4
E4 = ml_dtypes.float8_e4m3

B, S, D_IN, D = 4, 4096, 768, 128
N_DIN = D_IN // 128  # 6
TQ = 512             # queries per q-tile
NQ = S // 2          # queries per core
N_QT = NQ // TQ      # 4 q-tiles
SCALE = 1.0 / np.sqrt(np.float32(D))
CSHIFT = 2.0         # exp(s*SCALE - CSHIFT); num/den ratio is C-invariant
AF = mybir.ActivationFunctionType
DR = mybir.MatmulPerfMode.DoubleRow
N_WARM = 6

# ---------------------------------------------------------------------------
# Workarounds: the walrus build in this container accepts only ONE sync-wait
# command per instruction. TileContext's exit drain carries one wait per
# active proc, and Tile's sem assignment emits multi-wait instructions.
# Split both onto single-wait carrier instructions.
# ---------------------------------------------------------------------------


def _split_drain_and_barrier(self, tick_clock, wait_clock):
    gc = tick_clock.global_clock
    engs = [self.nc.sync, self.nc.scalar, self.nc.vector, self.nc.tensor]
    k = 0
    for p in range(N_PROCS):
        if gc[p] == 0:
            continue
        vc = VectorClock([gc[q] if q == p else 0 for q in range(N_PROCS)])
        d = engs[k % len(engs)].drain()
        k += 1
        wait_clock.add_sem_waits(d.ins, ScopedClock({None: vc}))
    self.nc.all_engine_barrier()
    assert self.sems is not None
    popped = self.nc._tile_sem_poison_stack.pop()
    assert popped is self._sem_poison
    self.nc.clear_and_free_semaphores(list(self.sems.allocated().values()))
    self.nc.all_engine_barrier()


tile_mod.TileContext._drain_and_barrier = _split_drain_and_barrier


def _split_waits(nc, max_waits=1):
    for fn in nc.m.functions:
        for bb in fn.blocks:
            insts = bb.instructions
            if not any(
                i.sync_info and i.sync_info.on_wait
                and len(i.sync_info.on_wait) > max_waits
                for i in insts
            ):
                continue
            new = []
            for inst in insts:
                si = inst.sync_info
                ow = list(si.on_wait) if si and si.on_wait else []
                if len(ow) > max_waits:
                    excess, keep = ow[:-max_waits], ow[-max_waits:]
                    for j, w in enumerate(excess):
                        new.append(
                            mybir.InstEventSemaphore(
                                name=f"{inst.name}-wsplit{j}",
                                engine=inst.engine,
                                ins=[],
                                outs=[],
                                sync_info=mybir.SyncInfo(
                                    on_wait=[w], on_update=[]
                                ),
                            )
                        )
                    inst.sync_info = mybir.SyncInfo(
                        on_wait=keep, on_update=list(si.on_update or [])
                    )
                new.append(inst)
            bb.instructions = new


# ---------------------------------------------------------------------------
# Device program
# ---------------------------------------------------------------------------


def _build():
    nc = bass.Bass()
    x8 = nc.declare_dram_parameter("x8", [D_IN, S], f8, isOutput=False)
    xh = nc.declare_dram_parameter("xh", [D_IN, 512], f16, isOutput=False)
    W8 = nc.declare_dram_parameter("W8", [128, 9 * 256], f8, isOutput=False)
    Wh = nc.declare_dram_parameter("Wh", [128, N_DIN * 3 * D], f16,
                                   isOutput=False)
    mask = nc.declare_dram_parameter("mask", [128, 4 * 128], f16,
                                     isOutput=False)
    out_num = nc.declare_dram_parameter("out_num", [D, NQ], f16, isOutput=True)
    out_den = nc.declare_dram_parameter("out_den", [1, NQ], f32, isOutput=True)

    with tile_mod.TileContext(nc) as tc:
        with (
            tc.tile_pool(name="persist", bufs=1) as persist,
            tc.tile_pool(name="work", bufs=6) as work,
            tc.tile_pool(name="outp", bufs=2) as outp,
            tc.tile_pool(name="ps_big", bufs=2, space="PSUM") as ps_big,
            tc.tile_pool(name="ps_out", bufs=1, space="PSUM") as ps_out,
            tc.tile_pool(name="ps_sml", bufs=2, space="PSUM") as ps_sml,
            tc.tile_pool(name="ps_den", bufs=1, space="PSUM") as ps_den,
        ):
            x_all8 = persist.tile([128, N_DIN, S], f8, tag="x_all8")
            xh_all = persist.tile([128, N_DIN, 512], f16, tag="xh_all")
            w8_all = persist.tile([128, 9, 2, 128], f8, tag="w8_all")
            wh_all = persist.tile([128, N_DIN * 3 * D], f16, tag="wh_all")
            m_all = persist.tile([128, 4 * 128], f16, tag="m_all")
            kt_sb = [persist.tile([128, 512], f16, tag=f"kt{c}", name=f"kt{c}")
                     for c in range(S // 512)]
            qt_sb = [persist.tile([128, TQ], f16, tag=f"qt{t}", name=f"qt{t}")
                     for t in range(N_QT)]
            # v8_sb[g][p, j, d] = v[key 128*(4g+j)+p, d] in fp8
            v8_sb = [persist.tile([128, 4, 128], f8, tag=f"v{g}", name=f"v{g}")
                     for g in range(S // 512)]
            # exact f16 v for key blocks 0,1 (own) and 16,17 (other)
            v16h = [persist.tile([128, 2, 128], f16, tag=f"vh{i}",
                                 name=f"vh{i}") for i in range(2)]
            # dedicated diag pt tiles; masked heads zeroed once, then only
            # the live region is ever rewritten -> full-width DR AV/den safe.
            # types: 0=A-own(los 0,128) 1=B-own(256,384) 2=A-oth 3=B-oth
            ptd = [persist.tile([128, 2, TQ], f8, tag=f"ptd{i}",
                                name=f"ptd{i}") for i in range(4)]
            # f16 carve-out pt for t=0 pairs kp=0 / kp=16
            pt16 = [persist.tile([128, 2, TQ], f16, tag=f"pt16_{i}",
                                 name=f"pt16_{i}") for i in range(2)]
            ones8 = persist.tile([128, 2, 128], f8, tag="ones8")
            ones16 = persist.tile([128, 128], f16, tag="ones16")
            biasC = persist.tile([128, 1], f32, tag="biasC")
            warm_sb = persist.tile([128, 1024], f16, tag="warm")

            # W8 host layout: [K|V|Q] blocks, 3 di-pairs each: idx = 3*b+m
            def w8_k(m):
                return w8_all[:, 0 + m]

            def w8_v(m):
                return w8_all[:, 3 + m]

            def w8_q(m):
                return w8_all[:, 6 + m]

            # Wh host layout: [K|V|Q], di-major inside
            def wh_k(di):
                return wh_all[:, 128 * di:128 * (di + 1)]

            def wh_v(di):
                return wh_all[:, 768 + 128 * di:768 + 128 * (di + 1)]

            def wh_q(di):
                return wh_all[:, 1536 + 128 * di:1536 + 128 * (di + 1)]

            tri = [m_all[:, 0:128], m_all[:, 128:256]]  # half1, half2
            ident = m_all[:, 256:384]
            fullm = m_all[:, 384:512]  # all -1000

            # input DMAs. sync: W8 + mask (+ per-tile outputs later);
            # vector: xh + Wh (head projections are the first real PE work);
            # gpsimd: x8 column waves (per-di for wave(0,0)).
            # sync ring (priority FIFO): head + weights first, then the
            # later x8 waves. gpsimd ring: memsets only.
            nc.sync.dma_start(out=xh_all[:], in_=xh.rearrange(
                "(d p) c -> p d c", p=128))
            nc.sync.dma_start(out=wh_all[:], in_=Wh[:])
            nc.sync.dma_start(out=w8_all[:], in_=W8.rearrange(
                "p (i a b) -> p i a b", i=9, a=2))
            nc.sync.dma_start(out=m_all[:], in_=mask[:])
            xsrc = x8.rearrange("(d p) c -> p d c", p=128)
            half = S // 2

            def x_wave(t, h, eng):
                lo = 512 * t + half * h
                eng.dma_start(
                    out=x_all8[:, :, lo:lo + 512], in_=xsrc[:, :, lo:lo + 512]
                )

            nc.gpsimd.memset(warm_sb[:], 0.0)
            for di in range(N_DIN):  # wave(0,0) per-di
                nc.sync.dma_start(
                    out=x_all8[:, di, 0:512], in_=xsrc[:, di, 0:512]
                )
            x_wave(0, 1, nc.sync)
            for t in (1, 2, 3):
                x_wave(t, 0, nc.sync)
                x_wave(t, 1, nc.sync)
            nc.gpsimd.memset(biasC[:], -float(CSHIFT))
            # B-diag pt heads [0:256) stay permanently zero (exp only ever
            # writes [256:512)); A-diag pairs write their full range
            for i in (1, 3):
                nc.gpsimd.memset(ptd[i][:, :, 0:256], 0.0)
            nc.gpsimd.memset(ones8[:], 1.0)
            nc.gpsimd.memset(ones16[:], 1.0)

            # PE pre-warm bridging the input-DMA wait (HAM + pstate
            # ramp). 1024-col matmuls into the (preamble-idle) score psum
            # pool keep duty high despite the 2-buf WAR rotation.
            for i in range(N_WARM):
                psw = ps_big.tile([128, 2, TQ], f32, tag="big",
                                  name=f"warm{i}")
                for s_ in (0, 1):
                    nc.tensor.matmul(
                        psw[:, s_, :], lhsT=warm_sb[:, 0:128],
                        rhs=warm_sb[:, 512 * s_:512 * (s_ + 1)],
                        start=True, stop=True,
                    )

            def x8_cols(m, c0, c1):
                return x_all8[:, 2 * m:2 * m + 2, c0:c1]

            # ---- head (f16-exact) projections: deps only on xh + Wh ----
            def project_heads():
                # kt0[:,0:256], kt4[:,0:256], qt0[:,0:256] and v16h from the
                # f16 head (own rows 0:256 = xh cols 0:256, other = 256:512)
                for dst, wsel, hcol in (
                    (kt_sb[0], wh_k, 0), (kt_sb[4], wh_k, 256),
                    (qt_sb[0], wh_q, 0),
                ):
                    ps = ps_sml.tile([128, 512], f32, tag="sml",
                                     name=f"ph{hcol}_{dst.name}")
                    for di in range(N_DIN):
                        nc.tensor.matmul(
                            ps[:, 0:256], lhsT=wsel(di),
                            rhs=xh_all[:, di, hcol:hcol + 256],
                            start=(di == 0), stop=(di == N_DIN - 1),
                        )
                    nc.vector.tensor_copy(dst[:, 0:256], ps[:, 0:256])
                for i in range(2):  # v16h own/other
                    ps = ps_sml.tile([128, 512], f32, tag="sml",
                                     name=f"phv{i}")
                    for j in range(2):
                        c0 = 256 * i + 128 * j
                        for di in range(N_DIN):
                            nc.tensor.matmul(
                                ps[:, 128 * j:128 * (j + 1)],
                                lhsT=xh_all[:, di, c0:c0 + 128],
                                rhs=wh_v(di),
                                start=(di == 0), stop=(di == N_DIN - 1),
                            )
                    nc.vector.tensor_copy(v16h[i][:], ps[:, 0:256])

            # ---- fp8 DoubleRow projections ----
            def project_kt8(c):
                lo = 256 if c in (0, 4) else 0  # head owns [0:256)
                ps = ps_sml.tile([128, 512], f32, tag="sml", name=f"pkt{c}")
                for m in range(3):
                    nc.tensor.matmul(
                        ps[:, lo:512], lhsT=w8_k(m),
                        rhs=x8_cols(m, 512 * c + lo, 512 * (c + 1)),
                        start=(m == 0), stop=(m == 2), perf_mode=DR,
                    )
                nc.vector.tensor_copy(kt_sb[c][:, lo:512], ps[:, lo:512])

            def project_qt8(t):
                lo = 256 if t == 0 else 0
                ps = ps_sml.tile([128, 512], f32, tag="sml", name=f"pqt{t}")
                for m in range(3):
                    nc.tensor.matmul(
                        ps[:, lo:512], lhsT=w8_q(m),
                        rhs=x8_cols(m, TQ * t + lo, TQ * (t + 1)),
                        start=(m == 0), stop=(m == 2), perf_mode=DR,
                    )
                nc.vector.tensor_copy(qt_sb[t][:, lo:512], ps[:, lo:512])

            def project_v8_group(g):
                ps = ps_sml.tile([128, 512], f32, tag="sml", name=f"pv{g}")
                for j in range(4):
                    k = 4 * g + j
                    for m in range(3):
                        nc.tensor.matmul(
                            ps[:, 128 * j:128 * (j + 1)],
                            lhsT=x8_cols(m, 128 * k, 128 * (k + 1)),
                            rhs=w8_v(m),
                            start=(m == 0), stop=(m == 2), perf_mode=DR,
                        )
                nc.vector.tensor_copy(v8_sb[g][:], ps[:])

            def proj_ops(t):
                """Projection op closures for q-tile t (emitted one tile
                early). For the last tile some projections are deferred into
                its own pair loop (late_ops) as PE filler under the final
                exp chain."""
                if t >= N_QT:
                    return []
                if t == N_QT - 1:
                    return [
                        lambda: project_qt8(t),
                        lambda: project_kt8(t),
                    ]
                return [
                    lambda: project_kt8(t),
                    lambda: project_kt8(N_QT + t),
                    lambda: project_v8_group(t),
                    lambda: project_v8_group(N_QT + t),
                    lambda: project_qt8(t),
                ]

            def fill_ops(t):
                if t == 0:
                    return [
                        lambda: project_v8_group(0),
                        lambda: project_kt8(N_QT),
                        lambda: project_v8_group(N_QT),
                    ] + proj_ops(1)
                return proj_ops(t + 1)

            def late_ops(t):
                if t != N_QT - 1:
                    return []
                return [
                    (3, lambda: project_v8_group(t)),
                    (9, lambda: project_kt8(N_QT + t)),
                    (11, lambda: project_v8_group(N_QT + t)),
                ]

            # ---- preamble projection schedule: only what the first two
            # score emits need; the rest becomes tile-0 fill work placed
            # after the score emits that precede their first consumers ----
            project_heads()
            project_kt8(0)
            project_qt8(0)

            n_kt_half = NQ // 128  # 16

            for t in range(N_QT):
                pairs = [2 * j for j in range(2 * (t + 1))] + [
                    n_kt_half + 2 * j for j in range(2 * (t + 1))
                ]
                n = len(pairs)

                def pair_info(kp):
                    half2 = kp >= n_kt_half
                    rel = kp - n_kt_half if half2 else kp
                    diag = 4 * t <= rel < 4 * t + 4
                    los = (
                        [128 * (rel - 4 * t), 128 * (rel - 4 * t + 1)]
                        if diag else [0, 0]
                    )
                    carve = t == 0 and rel == 0
                    return half2, diag, los, carve

                def emit_scores(kp, name):
                    # non-carve diag pairs: both s_ cover [lo0:512) so exp is
                    # a single 3D slice; s_=1's extra 128 cols get a full
                    # -1000 mask (exp -> 0). carve keeps per-s_ ranges.
                    half2, diag, los, carve = pair_info(kp)
                    ps = ps_big.tile([128, 2, TQ], f32, tag="big", name=name)
                    for s_ in (0, 1):
                        kt = kp + s_
                        lo = los[s_] if (carve or not diag) else los[0]
                        nc.tensor.matmul(
                            ps[:, s_, lo:TQ],
                            lhsT=kt_sb[kt // 4][:, 128 * (kt % 4):128 * (kt % 4 + 1)],
                            rhs=qt_sb[t][:, lo:TQ],
                            start=True,
                            stop=not diag,
                            skip_group_check=diag,
                        )
                        if diag:
                            if s_ == 1 and not carve:
                                nc.tensor.matmul(
                                    ps[:, s_, lo:lo + 128],
                                    lhsT=ident,
                                    rhs=fullm,
                                    start=False,
                                    stop=False,
                                    skip_group_check=True,
                                )
                            nc.tensor.matmul(
                                ps[:, s_, los[s_]:los[s_] + 128],
                                lhsT=ident,
                                rhs=tri[1 if half2 else 0],
                                start=False,
                                stop=True,
                                skip_group_check=True,
                            )
                    return ps

                # software pipeline: scores one pair ahead of exp/AV
                fill = fill_ops(t)
                fill_done = 0
                late = list(late_ops(t))

                po = ps_out.tile([128, TQ], f32, tag="out", name=f"po{t}")
                dn = ps_den.tile([128, TQ], f32, tag="den", name=f"dn{t}")

                fa = [True]
                fd = [True]
                sacc_on = [False]
                odd_ctr = [0]
                sacc = outp.tile([128, TQ], f16, tag="sacc", name=f"sacc{t}")

                def emit_avden(kp, pt, last):
                    half2, diag, los, carve = pair_info(kp)
                    g, j = kp // 4, kp % 4
                    if carve:
                        vh = v16h[1 if half2 else 0]
                        for s_ in (0, 1):
                            lo = los[s_]
                            nc.tensor.matmul(
                                po[:, lo:TQ], lhsT=vh[:, s_, :],
                                rhs=pt[:, s_, lo:TQ],
                                start=fa[0], stop=False,
                                skip_group_check=True,
                            )
                            fa[0] = False
                            nc.tensor.matmul(
                                dn[:, lo:TQ], lhsT=ones16[:],
                                rhs=pt[:, s_, lo:TQ],
                                start=fd[0], stop=False,
                                skip_group_check=True,
                            )
                            fd[0] = False
                    else:
                        nc.tensor.matmul(
                            po[:], lhsT=v8_sb[g][:, j:j + 2, :], rhs=pt[:],
                            start=fa[0], stop=last,
                            perf_mode=DR, skip_group_check=True,
                        )
                        fa[0] = False
                        take_dve = False
                        if not diag:
                            take_dve = odd_ctr[0] % 2 == 1
                            odd_ctr[0] += 1
                        if take_dve:
                            # every 2nd off-diag pair: den partial sums on
                            # the (lightly loaded) DVE, folded into dn once
                            # per tile by a single f16 matmul
                            if not sacc_on[0]:
                                nc.vector.tensor_copy(sacc[:], pt[:, 0, :])
                                sacc_on[0] = True
                            else:
                                nc.vector.tensor_add(
                                    sacc[:], sacc[:], pt[:, 0, :])
                            nc.vector.tensor_add(sacc[:], sacc[:], pt[:, 1, :])
                        else:
                            nc.tensor.matmul(
                                dn[:], lhsT=ones8[:], rhs=pt[:],
                                start=fd[0], stop=last and not sacc_on[0],
                                perf_mode=DR, skip_group_check=True,
                            )
                            fd[0] = False

                def emit_exp(kp, ps, name):
                    half2, diag, los, carve = pair_info(kp)
                    if carve:
                        pt = pt16[1 if half2 else 0]
                        for s_ in (0, 1):
                            lo = los[s_]
                            nc.scalar.activation(
                                pt[:, s_, lo:TQ], ps[:, s_, lo:TQ],
                                AF.Exp, scale=float(SCALE), bias=biasC[:],
                            )
                    elif diag and los[0]:
                        # B-diag: dedicated tile, head [0:256) stays 0
                        pt = ptd[3 if half2 else 1]
                        lo0 = los[0]
                        nc.scalar.activation(
                            pt[:, :, lo0:TQ], ps[:, :, lo0:TQ],
                            AF.Exp, scale=float(SCALE), bias=biasC[:],
                        )
                    else:
                        # off-diag and A-diag: full-width single exp
                        pt = work.tile([128, 2, TQ], f8, tag="pt", name=name)
                        nc.scalar.activation(
                            pt[:], ps[:], AF.Exp, scale=float(SCALE),
                            bias=biasC[:],
                        )
                    return pt

                while late and late[0][0] <= 0:
                    late.pop(0)[1]()
                ps_q = [emit_scores(pairs[0], f"s{t}_0")]
                pend = []  # (kp, pt) with AV/den deferred by one step
                for i, kp in enumerate(pairs):
                    if i + 1 < n:
                        ps_q.append(emit_scores(pairs[i + 1], f"s{t}_{i + 1}"))
                    # AV/den run two pairs behind their exp: hides the exp
                    # latency and the po/dn WAR at tile boundaries
                    if len(pend) >= 2:
                        pkp, ppt = pend.pop(0)
                        emit_avden(pkp, ppt, False)
                    want = ((i + 1) * len(fill)) // n
                    while fill_done < want:
                        fill[fill_done]()
                        fill_done += 1
                    while late and late[0][0] <= i + 1:
                        late.pop(0)[1]()
                    ps = ps_q.pop(0)
                    pt = emit_exp(kp, ps, f"p{t}_{kp}")
                    pend.append((kp, pt))
                while pend:
                    pkp, ppt = pend.pop(0)
                    emit_avden(pkp, ppt, not pend)
                if sacc_on[0]:
                    nc.tensor.matmul(
                        dn[:], lhsT=ones16[:], rhs=sacc[:],
                        start=False, stop=True, skip_group_check=True,
                    )
                ob = outp.tile([128, TQ], f16, tag="ob", name=f"ob{t}")
                db = outp.tile([1, TQ], f32, tag="db", name=f"db{t}")
                nc.vector.tensor_copy(ob[:], po[:])
                nc.vector.tensor_copy(db[:], dn[0:1, :])
                nc.sync.dma_start(out=out_num[:, TQ * t:TQ * (t + 1)], in_=ob[:])
                nc.sync.dma_start(
                    out=out_den[:, TQ * t:TQ * (t + 1)], in_=db[:]
                )
    _split_waits(nc)
    return nc


_NC_CACHE = []


def _get_nc():
    if not _NC_CACHE:
        _NC_CACHE.append(_build())
    return _NC_CACHE[0]


def _host_inputs(x, Wq, Wk, Wv):
    # W8 layout: [K|V|Q] blocks, each [128, 3 pairs, 2, 128] -> [128, 2304]
    def blk8(M):
        return (M.astype(np.float32).reshape(3, 2, 128, D)
                .transpose(2, 0, 1, 3).reshape(128, 768))

    W8 = np.ascontiguousarray(
        np.concatenate([blk8(Wk), blk8(Wv), blk8(Wq)], axis=1)
    ).astype(E4)

    # Wh layout: [K|V|Q], di-major inside (f16)
    def blkh(M):
        return M.astype(np.float16).reshape(N_DIN, 128, D).transpose(1, 0, 2)

    Wh = np.ascontiguousarray(
        np.concatenate([blkh(Wk), blkh(Wv), blkh(Wq)], axis=1)
        .reshape(128, N_DIN * 3 * D)
    )
    u = np.arange(128)[:, None]
    i = np.arange(128)[None, :]
    masks = {}
    for h in (0, 1):
        tri1 = (u <= i).astype(np.float32)          # own-parity half
        tri2 = (u <= i - 1 + h).astype(np.float32)  # other-parity half
        ma = np.concatenate(
            [(tri1 - 1.0) * 1000.0, (tri2 - 1.0) * 1000.0,
             np.eye(128, dtype=np.float32),
             np.full((128, 128), -1000.0, dtype=np.float32)], axis=1
        )
        masks[h] = np.ascontiguousarray(ma).astype(np.float16)
    in_maps = []
    for c in range(2 * B):
        b, h = divmod(c, 2)
        xp = np.concatenate([x[b, h::2], x[b, 1 - h::2]], axis=0)  # [S, 768]
        x8_p = np.ascontiguousarray(xp.T).astype(E4)  # [768, S]
        xh_p = np.ascontiguousarray(
            np.concatenate([xp[0:256], xp[2048:2304]], axis=0).T
        ).astype(np.float16)  # [768, 512]
        in_maps.append({"x8": x8_p, "xh": xh_p, "W8": W8, "Wh": Wh,
                        "mask": masks[h]})
    return in_maps


def kernel(x, Wq, Wk, Wv):
    x = np.asarray(x, np.float32)
    Wq = np.asarray(Wq, np.float32)
    Wk = np.asarray(Wk, np.float32)
    Wv = np.asarray(Wv, np.float32)
    nc = _get_nc()
    in_maps = _host_inputs(x, Wq, Wk, Wv)
    res = run_bass_kernel_spmd(nc, in_maps, list(range(2 * B)))
    out = np.empty((B, S, D), np.float32)
    for c in range(2 * B):
        b, h = divmod(c, 2)
        num = res.results[c]["out_num"].astype(np.float32)  # [128, NQ]
        den = res.results[c]["out_den"][0]       # [NQ] f32
        out[b, h::2, :] = (num / den[None, :]).T
    return out
